# revision 1
# baseline (speedup 1.0000x reference)
"""Trainium2 Bass kernel for a dense transformer block (nn_Block_88338887344891).

Distribution over 8 NeuronCores (single SPMD NEFF, 3 collectives):
  - LayerNorm1 token-sharded (512 tokens/core, feature-major) -> AllGather h^T
  - QKV projection head-sharded (2 heads/core), causal attention per (batch, head)
  - attention output AllToAll per head (head-shard -> token-shard), overlapped
  - output projection + residual + LN2 + full MLP token-sharded (no collective)

All matmuls run as float32r (TF32 mantissa, fp32 accumulate) at 1 cycle/row.
Activations are feature-major ([D on partitions, tokens free]) so per-token
LayerNorm/softmax reductions are ones-matmuls on the PE and per-feature params
are natural per-partition scalars. The two heads of a core are stacked on
partitions 0-63 / 64-127; score matmuls for h0/h1 then occupy disjoint PE
row-groups (auto tile_position) and run concurrently, and one Exp covers both.
"""
import numpy as np
from contextlib import ExitStack

try:  # persistent XLA cache so repeat runs skip the NEFF compile
    import jax
    jax.config.update("jax_compilation_cache_dir", "/tmp/jax_neff_cache")
    jax.config.update("jax_persistent_cache_min_compile_time_secs", 1.0)
except Exception:
    pass

import concourse.bass as bass
import concourse.bacc as bacc
import concourse.tile as tile
import concourse.mybir as mybir
from concourse.masks import make_identity
from concourse import bass_utils

AF = mybir.ActivationFunctionType
ALU = mybir.AluOpType
F32 = mybir.dt.float32
F32R = mybir.dt.float32r

NC_N = 8          # cores
B, T, D, H = 2, 2048, 1024, 16
HD = D // H       # 64
DFF = 4 * D       # 4096
EPS = 1e-5
TPC = (B * T) // NC_N    # 512 tokens per core
HPC = H // NC_N          # 2 heads per core
PO = D // 128            # 8 D-tiles
M1 = DFF // 128          # 32 ff1 out tiles
C_GELU = float(np.sqrt(2.0 / np.pi))
RG = [list(range(NC_N))]

# Native ACT gelu table (1 op) for hardware; CoreSim doesn't implement it,
# so sim runs (test.py --sim / analyze.py) flip this to the composed form.
GELU_NATIVE = True

_CACHE = {}


def _build():
    nc = bacc.Bacc("TRN2", target_bir_lowering=False, debug=False,
                   num_devices=NC_N)

    # ---- per-core external inputs ----
    xt_in = nc.dram_tensor("xt", [D, TPC], F32R, kind="ExternalInput")
    ln1w_in = nc.dram_tensor("ln1w", [128, PO], F32, kind="ExternalInput")
    ln1b_in = nc.dram_tensor("ln1b", [128, PO], F32, kind="ExternalInput")
    ln2w_in = nc.dram_tensor("ln2w", [128, PO], F32, kind="ExternalInput")
    ln2b_in = nc.dram_tensor("ln2b", [128, PO], F32, kind="ExternalInput")
    wqk_in = nc.dram_tensor("wqk", [D, 256], F32R, kind="ExternalInput")
    bqk_in = nc.dram_tensor("bqk", [128, 2], F32, kind="ExternalInput")
    wv_in = nc.dram_tensor("wv", [D, 128], F32R, kind="ExternalInput")
    bv_in = nc.dram_tensor("bv", [128, 1], F32, kind="ExternalInput")
    wo_in = nc.dram_tensor("wo", [PO, D, 128], F32R, kind="ExternalInput")
    bo_in = nc.dram_tensor("bo", [128, PO], F32, kind="ExternalInput")
    wf1_in = nc.dram_tensor("wf1", [M1, D, 128], F32R, kind="ExternalInput")
    bf1_in = nc.dram_tensor("bf1", [128, M1], F32, kind="ExternalInput")
    wf2_in = nc.dram_tensor("wf2", [PO, DFF, 128], F32R, kind="ExternalInput")
    bf2_in = nc.dram_tensor("bf2", [128, PO], F32, kind="ExternalInput")
    out_t = nc.dram_tensor("outt", [D, TPC], F32, kind="ExternalOutput")

    with tile.TileContext(nc) as tc, ExitStack() as ctx:
        perm = ctx.enter_context(tc.tile_pool(name="perm", bufs=1))
        big = ctx.enter_context(tc.tile_pool(name="big", bufs=1))
        psum = ctx.enter_context(tc.tile_pool(name="psum", bufs=4, space="PSUM"))
        ps2 = ctx.enter_context(tc.tile_pool(name="ps2", bufs=2, space="PSUM"))
        rows = ctx.enter_context(tc.tile_pool(name="rows", bufs=1))
        sqp = ctx.enter_context(tc.tile_pool(name="sqp", bufs=2))
        dram = ctx.enter_context(tc.tile_pool(name="dram", bufs=1, space="DRAM"))

        # ---- constants ----
        ones_col_f = perm.tile([128, 1], F32)
        nc.vector.memset(ones_col_f[:], 1.0)
        ones_col_r = perm.tile([128, 1], F32R)
        nc.vector.tensor_copy(ones_col_r[:], ones_col_f[:])
        ones_row_f = perm.tile([1, 128], F32)
        nc.vector.memset(ones_row_f[:], 1.0)
        ones_row_r = perm.tile([1, 128], F32R)
        nc.vector.tensor_copy(ones_row_r[:], ones_row_f[:])
        ident = perm.tile([128, 128], F32)
        make_identity(nc, ident[:])

        def load_const(t_in, shape, tag):
            t = perm.tile(shape, F32, tag=tag)
            nc.sync.dma_start(t[:], t_in.ap())
            return t

        ln1w = load_const(ln1w_in, [128, PO], "c_ln1w")
        ln1b = load_const(ln1b_in, [128, PO], "c_ln1b")
        ln2w = load_const(ln2w_in, [128, PO], "c_ln2w")
        ln2b = load_const(ln2b_in, [128, PO], "c_ln2b")
        bqk = load_const(bqk_in, [128, 2], "c_bqk")
        bv = load_const(bv_in, [128, 1], "c_bv")
        bo = load_const(bo_in, [128, PO], "c_bo")
        bf1 = load_const(bf1_in, [128, M1], "c_bf1")
        bf2 = load_const(bf2_in, [128, PO], "c_bf2")

        X1 = big.tile([128, PO, TPC], F32R, tag="x1")
        nc.sync.dma_start(X1[:], xt_in.ap().rearrange("(po p) t -> p po t", p=128))

        def layernorm(X, w_sb, b_sb, Hout):
            """Feature-major LN over partition(D) axis; X, Hout [128, PO, TPC]."""
            ps_s = psum.tile([1, TPC], F32, tag="ps")
            for po in range(PO):
                nc.tensor.matmul(ps_s[:], ones_col_r[:], X[:, po, :],
                                 start=(po == 0), stop=(po == PO - 1))
            ps_q = psum.tile([1, TPC], F32, tag="ps")
            for po in range(PO):
                sq = sqp.tile([128, TPC], F32R, tag="sq")
                nc.vector.tensor_mul(sq[:], X[:, po, :], X[:, po, :])
                nc.tensor.matmul(ps_q[:], ones_col_r[:], sq[:],
                                 start=(po == 0), stop=(po == PO - 1))
            mu = rows.tile([1, TPC], F32R, tag="mu")
            nc.scalar.activation(mu[:], ps_s[:], AF.Copy, scale=1.0 / D)
            ex2 = rows.tile([1, TPC], F32, tag="ex2")
            nc.scalar.activation(ex2[:], ps_q[:], AF.Copy, scale=1.0 / D)
            var = rows.tile([1, TPC], F32, tag="var")
            nc.vector.tensor_mul(var[:], mu[:].bitcast(F32), mu[:].bitcast(F32))
            nc.vector.tensor_sub(var[:], ex2[:], var[:])
            nc.vector.tensor_scalar_add(var[:], var[:], EPS)
            rec = rows.tile([1, TPC], F32, tag="rec")
            nc.vector.reciprocal(rec[:], var[:])
            inv = rows.tile([1, TPC], F32R, tag="inv")
            nc.scalar.activation(inv[:], rec[:], AF.Sqrt)
            ps_mu = psum.tile([128, TPC], F32, tag="ps")
            nc.tensor.matmul(ps_mu[:], ones_row_r[:], mu[:], start=True, stop=True)
            ps_inv = psum.tile([128, TPC], F32, tag="ps")
            nc.tensor.matmul(ps_inv[:], ones_row_r[:], inv[:], start=True, stop=True)
            for po in range(PO):
                t1 = Hout[:, po, :]
                nc.vector.tensor_sub(t1, X[:, po, :], ps_mu[:])
                nc.vector.tensor_mul(t1, t1, ps_inv[:])
                nc.vector.tensor_scalar(
                    out=t1, in0=t1, scalar1=w_sb[:, po:po + 1],
                    scalar2=b_sb[:, po:po + 1], op0=ALU.mult, op1=ALU.add)

        # ---- Phase 1: LN1 + AllGather h^T ----
        agi = dram.tile([D, TPC], F32R)
        agg = dram.tile([NC_N, D, TPC], F32R, addr_space="Shared")
        H1 = big.tile([128, PO, TPC], F32R, tag="h12")
        layernorm(X1, ln1w, ln1b, H1)
        nc.sync.dma_start(agi[:].rearrange("(po p) t -> p po t", p=128), H1[:])
        nc.gpsimd.collective_compute(
            "AllGather", ALU.bypass, replica_groups=RG,
            ins=[agi[:].opt()], outs=[agg[:].opt()])

        # ---- Phase 2: QKV (head-sharded, heads stacked on partitions) ----
        with tc.tile_pool(name="attn", bufs=1) as attn:
            QT = attn.tile([128, NC_N, TPC], F32R)
            KT = attn.tile([128, NC_N, TPC], F32R)
            Vt = attn.tile([128, 32, HPC, 65], F32R)
            nc.vector.tensor_copy(Vt[:, :, :, 64:65],
                                  ones_col_f[:].to_broadcast([128, 32, HPC, 1]))

            with tc.tile_pool(name="wqkv", bufs=1) as wqkv, \
                 tc.tile_pool(name="hcp", bufs=2) as hcp, \
                 tc.tile_pool(name="vtp", bufs=2) as vtp:
                wqk_sb = wqkv.tile([128, PO, 256], F32R)
                nc.sync.dma_start(
                    wqk_sb[:], wqk_in.ap().rearrange("(po p) m -> p po m", p=128))
                wv_sb = wqkv.tile([128, PO, 128], F32R)
                nc.sync.dma_start(
                    wv_sb[:], wv_in.ap().rearrange("(po p) m -> p po m", p=128))
                for c in range(NC_N):
                    Hc = hcp.tile([128, PO, TPC], F32R, tag="hc")
                    nc.sync.dma_start(
                        Hc[:], agg[c].rearrange("(po p) t -> p po t", p=128))
                    for m, DST in ((0, QT), (1, KT)):
                        psqk = psum.tile([128, TPC], F32, tag="ps")
                        for po in range(PO):
                            nc.tensor.matmul(
                                psqk[:], wqk_sb[:, po, 128 * m:128 * m + 128],
                                Hc[:, po, :],
                                start=(po == 0), stop=(po == PO - 1))
                        nc.vector.tensor_scalar_add(
                            DST[:, c, :], psqk[:], bqk[:, m:m + 1])
                    psv = psum.tile([128, TPC], F32, tag="ps")
                    for po in range(PO):
                        nc.tensor.matmul(psv[:], wv_sb[:, po, :], Hc[:, po, :],
                                         start=(po == 0), stop=(po == PO - 1))
                    vt_t = vtp.tile([128, TPC], F32, tag="vtt")
                    nc.vector.tensor_scalar_add(vt_t[:], psv[:], bv[:])
                    for tt in range(4):
                        g = 4 * c + tt
                        pst = psum.tile([128, 128], F32, tag="ps")
                        nc.tensor.transpose(
                            pst[:], vt_t[:, 128 * tt:128 * tt + 128], ident[:])
                        for h in range(HPC):
                            nc.vector.tensor_copy(
                                Vt[:, g, h, 0:64], pst[:, 64 * h:64 * h + 64])

            # ---- Phase 3: causal attention per (head, batch) ----
            # both heads' scores packed in one 2-bank psum + one Exp; the
            # h0/h1 score matmuls hit disjoint PE row groups and overlap.
            a2ai = dram.tile([NC_N, 128, TPC], F32R)
            a2ao = dram.tile([NC_N, 128, TPC], F32R)
            with tc.tile_pool(name="ptp", bufs=1) as ptp, \
                 tc.tile_pool(name="avp", bufs=2) as avp:
                for b in range(B):
                    for j in range(4):
                        n_kt = 4 * j + 4
                        PT = ptp.tile([128, 16, 2 * TPC], F32R, tag="pt")
                        for i in range(n_kt):
                            pss = ps2.tile([128, 2 * TPC], F32, tag="ps2")
                            cb = 4 * b + i // 4
                            off = (i % 4) * 128
                            for h in range(HPC):
                                nc.tensor.matmul(
                                    pss[:, h * TPC:(h + 1) * TPC],
                                    KT[64 * h:64 * h + 64, cb, off:off + 128],
                                    QT[64 * h:64 * h + 64, 4 * b + j, :],
                                    start=True, stop=True)
                            nc.scalar.activation(PT[:, i, :], pss[:],
                                                 AF.Exp, scale=0.125)
                            if i >= 4 * j:
                                nc.gpsimd.affine_select(
                                    out=PT[:, i, :].rearrange(
                                        "p (h q) -> p h q", h=HPC),
                                    in_=PT[:, i, :].rearrange(
                                        "p (h q) -> p h q", h=HPC),
                                    compare_op=ALU.is_ge, fill=0.0,
                                    base=-128 * (i - 4 * j),
                                    pattern=[[0, HPC], [1, TPC]],
                                    channel_multiplier=-1)
                        for h in range(HPC):
                            ps_av = psum.tile([65, TPC], F32, tag="ps")
                            for i in range(n_kt):
                                nc.tensor.matmul(
                                    ps_av[:], Vt[:, 16 * b + i, h, :],
                                    PT[:, i, h * TPC:(h + 1) * TPC],
                                    start=(i == 0), stop=(i == n_kt - 1))
                            avs = avp.tile([65, TPC], F32R, tag="avs")
                            nc.vector.tensor_copy(avs[:], ps_av[:])
                            rec = avp.tile([1, TPC], F32, tag="avrec")
                            nc.vector.reciprocal(rec[:], avs[64:65, :].bitcast(F32))
                            recr = avp.tile([1, TPC], F32R, tag="avrecr")
                            nc.vector.tensor_copy(recr[:], rec[:])
                            ps_bc = psum.tile([64, TPC], F32, tag="ps")
                            nc.tensor.matmul(ps_bc[:], ones_row_r[:, 0:64],
                                             recr[:], start=True, stop=True)
                            avn = avp.tile([64, TPC], F32R, tag="avn")
                            nc.vector.tensor_mul(avn[:], avs[0:64, :], ps_bc[:])
                            nc.sync.dma_start(
                                a2ai[4 * b + j, 64 * h:64 * h + 64, :], avn[:])

            nc.gpsimd.collective_compute(
                "AllToAll", ALU.bypass, replica_groups=RG,
                ins=[a2ai[:].opt()], outs=[a2ao[:].opt()])

        # ---- Phase 4: output projection + residual ----
        x2p = ctx.enter_context(tc.tile_pool(name="x2p", bufs=1))
        X2 = x2p.tile([128, PO, TPC], F32R, tag="x2")
        with tc.tile_pool(name="avtp", bufs=1) as avtp, \
             tc.tile_pool(name="wop", bufs=2) as wop:
            AVt = avtp.tile([128, NC_N, TPC], F32R)
            nc.sync.dma_start(AVt[:], a2ao[:].rearrange("s p t -> p s t"))
            for m in range(PO):
                wom = wop.tile([128, PO, 128], F32R, tag="wom")
                nc.sync.dma_start(
                    wom[:], wo_in.ap()[m].rearrange("(po p) n -> p po n", p=128))
                ps_o = psum.tile([128, TPC], F32, tag="ps")
                for po in range(PO):
                    nc.tensor.matmul(ps_o[:], wom[:, po, :], AVt[:, po, :],
                                     start=(po == 0), stop=(po == PO - 1))
                nc.vector.tensor_scalar_add(X2[:, m, :], ps_o[:], bo[:, m:m + 1])
                nc.vector.tensor_add(X2[:, m, :], X2[:, m, :], X1[:, m, :])

        # ---- Phase 5: LN2 + MLP ----
        H2 = big.tile([128, PO, TPC], F32R, tag="h12")
        layernorm(X2, ln2w, ln2b, H2)

        with tc.tile_pool(name="ap", bufs=1) as ap_pool, \
             tc.tile_pool(name="w1p", bufs=3) as w1p, \
             tc.tile_pool(name="w2p", bufs=2) as w2p, \
             tc.tile_pool(name="gp", bufs=2) as gp, \
             tc.tile_pool(name="outp", bufs=2) as outp:
            A = ap_pool.tile([128, M1, TPC], F32R)
            for m in range(M1):
                w1m = w1p.tile([128, PO, 128], F32R, tag="w1")
                nc.sync.dma_start(
                    w1m[:], wf1_in.ap()[m].rearrange("(po p) n -> p po n", p=128))
                ps1 = psum.tile([128, TPC], F32, tag="ps")
                for po in range(PO):
                    nc.tensor.matmul(ps1[:], w1m[:, po, :], H2[:, po, :],
                                     start=(po == 0), stop=(po == PO - 1))
                if GELU_NATIVE:
                    nc.scalar.activation(A[:, m, :], ps1[:], AF.Gelu_apprx_tanh,
                                         bias=bf1[:, m:m + 1])
                else:
                    t0 = gp.tile([128, TPC], F32R, tag="g0")
                    nc.vector.tensor_scalar_add(t0[:], ps1[:], bf1[:, m:m + 1])
                    sq = gp.tile([128, TPC], F32R, tag="g1")
                    nc.vector.tensor_mul(sq[:], t0[:], t0[:])
                    nc.vector.tensor_scalar(out=sq[:], in0=sq[:],
                                            scalar1=0.044715, scalar2=1.0,
                                            op0=ALU.mult, op1=ALU.add)
                    nc.vector.tensor_mul(sq[:], sq[:], t0[:])
                    nc.scalar.activation(sq[:], sq[:], AF.Tanh, scale=C_GELU)
                    nc.vector.tensor_scalar(out=sq[:], in0=sq[:], scalar1=1.0,
                                            scalar2=0.5, op0=ALU.add,
                                            op1=ALU.mult)
                    nc.vector.tensor_mul(A[:, m, :], sq[:], t0[:])

            out_view = out_t.ap().rearrange("(po p) t -> p po t", p=128)
            for m in range(PO):
                w2m = w2p.tile([128, M1, 128], F32R, tag="w2")
                nc.sync.dma_start(
                    w2m[:], wf2_in.ap()[m].rearrange("(ko p) n -> p ko n", p=128))
                ps_2 = psum.tile([128, TPC], F32, tag="ps")
                for ko in range(M1):
                    nc.tensor.matmul(ps_2[:], w2m[:, ko, :], A[:, ko, :],
                                     start=(ko == 0), stop=(ko == M1 - 1))
                om = outp.tile([128, TPC], F32, tag="om")
                nc.vector.tensor_scalar_add(om[:], ps_2[:], bf2[:, m:m + 1])
                nc.vector.tensor_add(om[:], om[:], X2[:, m, :].bitcast(F32))
                nc.sync.dma_start(out_view[:, m, :], om[:])

    nc.compile()
    return nc


def _get_nc():
    key = ("nc", GELU_NATIVE)
    if key not in _CACHE:
        _CACHE[key] = _build()
    return _CACHE[key]


def _make_in_maps(inputs):
    x = np.asarray(inputs["x"], np.float32).reshape(B * T, D)
    W_qkv = np.asarray(inputs["W_qkv"], np.float32)
    b_qkv = np.asarray(inputs["b_qkv"], np.float32)
    W_o = np.asarray(inputs["W_o"], np.float32)
    b_o = np.asarray(inputs["b_o"], np.float32)
    W_ff1 = np.asarray(inputs["W_ff1"], np.float32)
    b_ff1 = np.asarray(inputs["b_ff1"], np.float32)
    W_ff2 = np.asarray(inputs["W_ff2"], np.float32)
    b_ff2 = np.asarray(inputs["b_ff2"], np.float32)

    def pcol(v):  # [D'] -> [128, D'/128] feature-major per-partition layout
        return np.ascontiguousarray(v.reshape(-1, 128).T)

    common = {
        "ln1w": pcol(np.asarray(inputs["ln1_w"], np.float32)),
        "ln1b": pcol(np.asarray(inputs["ln1_b"], np.float32)),
        "ln2w": pcol(np.asarray(inputs["ln2_w"], np.float32)),
        "ln2b": pcol(np.asarray(inputs["ln2_b"], np.float32)),
        "wo": np.ascontiguousarray(W_o.reshape(D, PO, 128).transpose(1, 0, 2)),
        "bo": pcol(b_o),
        "wf1": np.ascontiguousarray(W_ff1.reshape(D, M1, 128).transpose(1, 0, 2)),
        "bf1": pcol(b_ff1),
        "wf2": np.ascontiguousarray(W_ff2.reshape(DFF, PO, 128).transpose(1, 0, 2)),
        "bf2": pcol(b_ff2),
    }
    in_maps = []
    for r in range(NC_N):
        hc = 128 * r          # first column of this core's Q/K/V head block
        m = dict(common)
        m["xt"] = np.ascontiguousarray(x[TPC * r:TPC * (r + 1)].T)
        m["wqk"] = np.ascontiguousarray(np.concatenate(
            [W_qkv[:, hc:hc + 128], W_qkv[:, D + hc:D + hc + 128]], axis=1))
        m["bqk"] = np.ascontiguousarray(np.stack(
            [b_qkv[hc:hc + 128], b_qkv[D + hc:D + hc + 128]], axis=1))
        m["wv"] = np.ascontiguousarray(W_qkv[:, 2 * D + hc:2 * D + hc + 128])
        m["bv"] = np.ascontiguousarray(b_qkv[2 * D + hc:2 * D + hc + 128]
                                       .reshape(128, 1))
        in_maps.append(m)
    return in_maps


def _run(inputs, trace=False, trace_cores=None):
    nc = _get_nc()
    in_maps = _make_in_maps(inputs)
    res = bass_utils.run_bass_kernel_spmd(
        nc, in_maps, core_ids=list(range(NC_N)), trace=trace,
        trace_cores=trace_cores)
    outs = [res.results[r]["outt"] for r in range(NC_N)]
    full = np.concatenate([o.T for o in outs], axis=0)
    return full.reshape(B, T, D).astype(np.float32), res


def kernel(**inputs):
    out, _ = _run(inputs, trace=False)
    return out



# revision 29
# speedup vs baseline: 1.9463x; 1.9463x over previous
"""Trainium2 Bass kernel for a dense transformer block (nn_Block_88338887344891).

Distribution over 8 NeuronCores (single SPMD NEFF, 2 collectives):
  - LN1 stats computed per-core on its own 512 tokens, AllGathered (48KB,
    ~16us, overlapped with the QKV matmuls on raw x).
  - LayerNorm is folded into the matmuls: QKV = inv * (W'^T x + s (-mu)) + b'
    where W' = diag(ln_w) W (host-folded), s = colsum(W'), so the heavy
    matmuls start before the stats arrive; ln_b is folded into b'.
  - QKV + causal attention head-sharded (2 heads/core over all 4096 tokens);
    scores/exp/AV trimmed to the causal region; V is produced directly in
    keys-major layout (x as the stationary operand) so no transposes.
  - attention output AllToAll per head in bf16 (1MB, ~41us)
  - output projection + residual + LN2 + full MLP token-sharded (512 tok/core)
  - gelu(tanh approx) via t*sigmoid(1.702 t) (1 ACT + 1 fused DVE op)
All matmul moving operands are bf16 (1 cycle/row); PSUM accumulates f32.
"""
import numpy as np
from contextlib import ExitStack

try:  # persistent XLA cache so repeat runs skip the NEFF compile
    import jax
    jax.config.update("jax_compilation_cache_dir", "/tmp/jax_neff_cache")
    jax.config.update("jax_persistent_cache_min_compile_time_secs", 1.0)
except Exception:
    pass

import ml_dtypes
import concourse.bass as bass
import concourse.bacc as bacc
import concourse.tile as tile
import concourse.mybir as mybir
from concourse import bass_utils

AF = mybir.ActivationFunctionType
ALU = mybir.AluOpType
F32 = mybir.dt.float32
F32R = mybir.dt.float32r
BF16 = mybir.dt.bfloat16
NPBF16 = ml_dtypes.bfloat16

NC_N = 8          # cores
B, T, D, H = 2, 2048, 1024, 16
HD = D // H       # 64
DFF = 4 * D       # 4096
EPS = 1e-5
BT = B * T               # 4096 tokens
TPC = BT // NC_N         # 512 tokens per core
HPC = H // NC_N          # 2 heads per core
PO = D // 128            # 8 D-tiles
M1 = DFF // 128          # 32 ff1 out tiles
NKT = BT // 128          # 32 key tiles globally (16 per batch)
RG = [list(range(NC_N))]
SIGC = 1.702             # gelu sigmoid-form constant

GELU_NATIVE = False   # kept for test.py compat; kernel is sim/hw identical

_CACHE = {}


def _build():
    nc = bacc.Bacc("TRN2", target_bir_lowering=False, debug=False,
                   num_devices=NC_N)

    # ---- per-core external inputs ----
    xb_in = nc.dram_tensor("xb", [D, BT], BF16, kind="ExternalInput")
    xs_in = nc.dram_tensor("xs", [D, TPC], BF16, kind="ExternalInput")
    xf_in = nc.dram_tensor("xf", [D, TPC], F32, kind="ExternalInput")
    wqkv_in = nc.dram_tensor("wqkv", [D, 384], BF16, kind="ExternalInput")
    sqkv_in = nc.dram_tensor("sqkv", [1, 256], F32R, kind="ExternalInput")
    svb_in = nc.dram_tensor("svb", [2, 128], BF16, kind="ExternalInput")
    bqkv_in = nc.dram_tensor("bqkv", [128, 2], F32, kind="ExternalInput")
    wo_in = nc.dram_tensor("wo", [D, D], BF16, kind="ExternalInput")
    bo_in = nc.dram_tensor("bo", [128, PO], F32, kind="ExternalInput")
    wf1_in = nc.dram_tensor("wf1", [D, DFF], BF16, kind="ExternalInput")
    bf1_in = nc.dram_tensor("bf1", [128, M1], F32, kind="ExternalInput")
    bf1s_in = nc.dram_tensor("bf1s", [128, M1], F32, kind="ExternalInput")
    wf2_in = nc.dram_tensor("wf2", [DFF, D], BF16, kind="ExternalInput")
    bf2_in = nc.dram_tensor("bf2", [128, PO], F32, kind="ExternalInput")
    out_t = nc.dram_tensor("outt", [D, TPC], F32, kind="ExternalOutput")

    with tile.TileContext(nc) as tc, ExitStack() as ctx:
        perm = ctx.enter_context(tc.tile_pool(name="perm", bufs=1))
        big = ctx.enter_context(tc.tile_pool(name="big", bufs=1))
        rows = ctx.enter_context(tc.tile_pool(name="rows", bufs=1))
        dram = ctx.enter_context(tc.tile_pool(name="dram", bufs=1, space="DRAM"))

        # ---- constants ----
        ones_col_b = perm.tile([128, 1], BF16)
        nc.vector.memset(ones_col_b[:], 1.0)
        ones_col_f = perm.tile([128, 1], F32)
        nc.vector.memset(ones_col_f[:], 1.0)
        ones_col_r = perm.tile([128, 1], F32R)
        nc.vector.tensor_copy(ones_col_r[:], ones_col_f[:])
        ones_row_f = perm.tile([1, 128], F32)
        nc.vector.memset(ones_row_f[:], 1.0)
        ones_row_r = perm.tile([1, 128], F32R)
        nc.vector.tensor_copy(ones_row_r[:], ones_row_f[:])
        ones_row_b = perm.tile([1, 128], BF16)
        nc.vector.tensor_copy(ones_row_b[:], ones_row_f[:])

        def load_const(t_in, shape, tag, dt=F32):
            t = perm.tile(shape, dt, tag=tag)
            nc.sync.dma_start(t[:], t_in.ap())
            return t

        sqkv = load_const(sqkv_in, [1, 256], "c_sqkv", F32R)
        sv_row = perm.tile([1, 128], BF16, tag="c_sv")
        nc.sync.dma_start(sv_row[:], svb_in.ap()[0:1])
        bv_row = perm.tile([1, 128], BF16, tag="c_bv")
        nc.sync.dma_start(bv_row[:], svb_in.ap()[1:2])
        bqkv = load_const(bqkv_in, [128, 2], "c_bqkv")
        bo = load_const(bo_in, [128, PO], "c_bo")
        bf1 = load_const(bf1_in, [128, M1], "c_bf1")
        bf1s = load_const(bf1s_in, [128, M1], "c_bf1s")
        bf2 = load_const(bf2_in, [128, PO], "c_bf2")

        # ---- persistent SBUF ----
        invc = big.tile([128, NKT], F32R, tag="invc")  # inv keys-major
        X2 = big.tile([128, PO, TPC], F32R, tag="x2")
        xh2 = big.tile([128, PO, TPC], BF16, tag="xh2")
        Amat = big.tile([128, M1, TPC], BF16, tag="amat")

        # dram scratch
        st_out = dram.tile([2, TPC], F32R)
        st_agg = dram.tile([NC_N, 2, TPC], F32R, addr_space="Shared")
        a2ai = dram.tile([NC_N, 128, TPC], BF16)
        a2ao = dram.tile([NC_N, 128, TPC], BF16)

        xb_view = xb_in.ap().rearrange("(po p) (k t) -> k p po t",
                                       p=128, t=TPC)

        xlp_cm = tc.tile_pool(name="xlp", bufs=1)
        xlp = xlp_cm.__enter__()
        xl = xlp.tile([128, PO, TPC], F32, tag="xl")   # my x (residual)
        wo_sb = xlp.tile([128, PO, D], BF16, tag="wo")
        qkvp_cm = tc.tile_pool(name="qkvp", bufs=1)
        qkvp = qkvp_cm.__enter__()
        Qh = qkvp.tile([128, BT], BF16, tag="qh")      # 2 heads stacked
        Kh = qkvp.tile([128, BT], BF16, tag="kh")
        Vt = qkvp.tile([128, HPC, NKT, 65], BF16, tag="vt")  # keys-major V
        invB = qkvp.tile([128, BT], F32, tag="invb")   # 1/std broadcast
        nc.gpsimd.memset(Vt[:, :, :, 64:65], 1.0)   # softmax denominator row

        # ============ Phase 1: local LN1 stats + AllGather ============
        with tc.tile_pool(name="wq", bufs=1) as wq, \
             tc.tile_pool(name="xsp", bufs=1) as xsp, \
             tc.tile_pool(name="xcp", bufs=2) as xcp, \
             tc.tile_pool(name="sqp", bufs=2) as sqp, \
             tc.tile_pool(name="strp", bufs=2) as strp, \
             tc.tile_pool(name="psA", bufs=6, space="PSUM") as psA, \
             tc.tile_pool(name="psV", bufs=2, space="PSUM") as psVt:
            wqkv_sb = wq.tile([128, PO, 384], BF16)
            nc.sync.dma_start(
                wqkv_sb[:], wqkv_in.ap().rearrange("(po p) m -> p po m", p=128))
            xst = xsp.tile([128, PO, TPC], BF16)
            nc.sync.dma_start(
                xst[:], xs_in.ap().rearrange("(po p) t -> p po t", p=128))

            # stats on my 512 tokens
            stmu_l = rows.tile([1, TPC], F32R, tag="stmu_l")
            stinv_l = rows.tile([1, TPC], F32R, tag="stinv_l")
            ps_s = psA.tile([128, TPC], F32, tag="ps")
            for po in range(PO):
                nc.tensor.matmul(ps_s[0:1, :], ones_col_b[:], xst[:, po, :],
                                 start=(po == 0), stop=(po == PO - 1))
            ps_q = psA.tile([128, TPC], F32, tag="ps")
            for po in range(PO):
                sq = sqp.tile([128, TPC], BF16, tag="sq")
                nc.vector.tensor_mul(sq[:], xst[:, po, :], xst[:, po, :])
                nc.tensor.matmul(ps_q[0:1, :], ones_col_b[:], sq[:],
                                 start=(po == 0), stop=(po == PO - 1))
            nc.scalar.activation(stmu_l[:].bitcast(F32), ps_s[0:1, :],
                                 AF.Copy, scale=-1.0 / D)     # -mu
            ex2 = rows.tile([1, TPC], F32, tag="ex2")
            nc.scalar.activation(ex2[:], ps_q[0:1, :], AF.Copy, scale=1.0 / D)
            mu2 = rows.tile([1, TPC], F32, tag="mu2")
            nc.vector.tensor_mul(mu2[:], stmu_l[:].bitcast(F32),
                                 stmu_l[:].bitcast(F32))
            var = rows.tile([1, TPC], F32, tag="var")
            nc.vector.scalar_tensor_tensor(
                out=var[:], in0=ex2[:], scalar=EPS, in1=mu2[:],
                op0=ALU.add, op1=ALU.subtract)
            rec = rows.tile([1, TPC], F32, tag="rec")
            nc.vector.reciprocal(rec[:], var[:])
            nc.scalar.activation(stinv_l[:].bitcast(F32), rec[:],
                                 AF.Sqrt)                      # inv = 1/std
            nc.sync.dma_start(st_out[0:1, :], stmu_l[:])
            nc.sync.dma_start(st_out[1:2, :], stinv_l[:])
            nc.gpsimd.collective_compute(
                "AllGather", ALU.bypass, replica_groups=RG,
                ins=[st_out[:].opt()], outs=[st_agg[:].opt()])
            # gathered stats (ACT dma queue: don't block SP chunk stream)
            st_agg_v = st_agg[:].rearrange("s r t -> r s t")
            # inv keys-major: invc[p, g] = inv[128*g + p]
            for s in range(NC_N):
                nc.scalar.dma_start(
                    invc[:, 4 * s:4 * s + 4],
                    st_agg[:].rearrange("s r (c p) -> s r p c", p=128)[s, 1])

            # ============ Phase 2: QKV over all chunks ============
            for c in range(NC_N):
                xc = xcp.tile([128, PO, TPC], BF16, tag="xc")
                nc.sync.dma_start(xc[:], xb_view[c])
                tok = slice(TPC * c, TPC * (c + 1))
                murow = strp.tile([1, TPC], F32R, tag="mur")
                nc.scalar.dma_start(murow[:], st_agg_v[0:1, c])
                invrow = strp.tile([1, TPC], F32R, tag="ivr")
                nc.scalar.dma_start(invrow[:], st_agg_v[1:2, c])
                mub = strp.tile([1, TPC], BF16, tag="mub")
                nc.vector.tensor_copy(mub[:], murow[:])
                psb = psA.tile([128, TPC], F32, tag="ps")
                nc.tensor.matmul(psb[:], ones_row_r[:], invrow[:],
                                 start=True, stop=True)
                nc.scalar.activation(invB[:, tok], psb[:], AF.Copy)
                for m, DST in ((0, Qh), (1, Kh)):
                    ps = psA.tile([128, TPC], F32, tag="ps")
                    for po in range(PO):
                        nc.tensor.matmul(
                            ps[:], wqkv_sb[:, po, 128 * m:128 * m + 128],
                            xc[:, po, :], start=(po == 0), stop=False)
                    nc.tensor.matmul(
                        ps[:], sqkv[:, 128 * m:128 * m + 128],
                        murow[:], start=False, stop=True)
                    nc.vector.tensor_mul(DST[:, tok], ps[:], invB[:, tok])
                    nc.vector.tensor_scalar_add(DST[:, tok], DST[:, tok],
                                                bqkv[:, m:m + 1])
                # V directly transposed: psv[key, dim], 4 key tiles / chunk
                for kt in range(4):
                    g = 4 * c + kt
                    ksl = slice(128 * kt, 128 * kt + 128)
                    psv = psVt.tile([128, 128], F32, tag="psv")
                    for po in range(PO):
                        nc.tensor.matmul(
                            psv[:], xc[:, po, ksl], wqkv_sb[:, po, 256:384],
                            start=(po == 0), stop=False)
                    nc.tensor.matmul(psv[:], mub[:, ksl], sv_row[:],
                                     start=False, stop=False)
                    nc.tensor.matmul(psv[:], ones_row_b[:], bv_row[:],
                                     start=False, stop=True)
                    for h in range(HPC):
                        nc.scalar.activation(
                            Vt[:, h, g, 0:64], psv[:, 64 * h:64 * h + 64],
                            AF.Copy, scale=invc[:, g:g + 1].bitcast(F32))

            # residual x + Wo weights: needed from phase 4 on
            nc.sync.dma_start(xl[:],
                              xf_in.ap().rearrange("(po p) t -> p po t", p=128))
            nc.sync.dma_start(
                wo_sb[:], wo_in.ap().rearrange("(po p) n -> p po n", p=128))

        # ============ Phase 3: causal attention ============
        with tc.tile_pool(name="ptp", bufs=3) as ptp, \
             tc.tile_pool(name="avp", bufs=2) as avp, \
             tc.tile_pool(name="psS", bufs=2, space="PSUM") as psS, \
             tc.tile_pool(name="psV2", bufs=2, space="PSUM") as psV2:
            for b in range(B):
                for h in range(HPC):
                    hsl = slice(64 * h, 64 * h + 64)
                    for qh in range(2):
                        n_i = 8 if qh == 0 else 16
                        ps_av = psV2.tile([65, 1024], F32, tag="psav")
                        for i in range(n_i):
                            koff = 2048 * b + 128 * i
                            qlo = max(1024 * qh, 128 * i)
                            free = 1024 * (qh + 1) - qlo
                            off = qlo - 1024 * qh
                            qabs = 2048 * b + qlo
                            # matmul outputs must stay inside one psum bank
                            # (512 f32): emit per-bank-half pieces, aligned
                            # to each psum tile's own banks.
                            pieces = []   # ps_av-aligned (offset off..1024)
                            lo = off
                            while lo < 1024:
                                hi = min(1024, (lo // 512 + 1) * 512)
                                pieces.append((lo, hi - lo))
                                lo = hi
                            spieces = []  # pss-aligned (offset 0..free)
                            lo = 0
                            while lo < free:
                                hi = min(free, (lo // 512 + 1) * 512)
                                spieces.append((lo, hi - lo))
                                lo = hi
                            pss = psS.tile([128, 1024], F32, tag="pss")
                            for (plo, pfree) in spieces:
                                qa = qabs + plo
                                nc.tensor.matmul(
                                    pss[:, plo:plo + pfree],
                                    Kh[hsl, koff:koff + 128],
                                    Qh[hsl, qa:qa + pfree],
                                    start=True, stop=True)
                            pt = ptp.tile([128, 1024], BF16, tag="pt")
                            nc.scalar.activation(pt[:, 0:free],
                                                 pss[:, 0:free],
                                                 AF.Exp, scale=0.125)
                            if 128 * i >= 1024 * qh:   # diagonal tile
                                nc.gpsimd.affine_select(
                                    out=pt[:, 0:128], in_=pt[:, 0:128],
                                    compare_op=ALU.is_ge, fill=0.0,
                                    base=0, pattern=[[1, 128]],
                                    channel_multiplier=-1)
                            for (plo, pfree) in pieces:
                                hb = plo // 512
                                last_i = 8 * qh + 3 if hb == 0 else n_i - 1
                                nc.tensor.matmul(
                                    ps_av[:, plo:plo + pfree],
                                    Vt[:, h, 16 * b + i, :],
                                    pt[:, plo - off:plo - off + pfree],
                                    start=(i == 0), stop=(i == last_i))
                        recd = rows.tile([1, 1024], F32R, tag="recd")
                        nc.vector.reciprocal(recd[:].bitcast(F32),
                                             ps_av[64:65, :])
                        ps_bc = psS.tile([128, 1024], F32, tag="pss")
                        for half in range(2):
                            nc.tensor.matmul(
                                ps_bc[0:64, 512 * half:512 * half + 512],
                                ones_row_r[:, 0:64],
                                recd[:, 512 * half:512 * half + 512],
                                start=True, stop=True)
                        avn = avp.tile([64, 1024], BF16, tag="avn")
                        nc.vector.tensor_mul(avn[:], ps_av[0:64, :],
                                             ps_bc[0:64, :])
                        g0 = 4 * b + 2 * qh
                        nc.sync.dma_start(a2ai[g0, hsl, :], avn[:, 0:TPC])
                        nc.sync.dma_start(a2ai[g0 + 1, hsl, :],
                                          avn[:, TPC:2 * TPC])

        nc.gpsimd.collective_compute(
            "AllToAll", ALU.bypass, replica_groups=RG,
            ins=[a2ai[:].opt()], outs=[a2ao[:].opt()])
        qkvp_cm.__exit__(None, None, None)   # free Qh/Kh/Vt/invB

        # ============ Phase 4: output projection + residual ============
        with tc.tile_pool(name="avtp", bufs=1) as avtp, \
             tc.tile_pool(name="psA2", bufs=6, space="PSUM") as psA2:
            AVt = avtp.tile([128, NC_N, TPC], BF16)
            nc.scalar.dma_start(AVt[:], a2ao[:].rearrange("s p t -> p s t"))
            for m in range(PO):
                ps = psA2.tile([128, TPC], F32, tag="ps")
                for po in range(PO):
                    nc.tensor.matmul(
                        ps[:], wo_sb[:, po, 128 * m:128 * m + 128],
                        AVt[:, po, :], start=(po == 0), stop=(po == PO - 1))
                nc.vector.scalar_tensor_tensor(
                    out=X2[:, m, :].bitcast(F32), in0=ps[:],
                    scalar=bo[:, m:m + 1], in1=xl[:, m, :],
                    op0=ALU.add, op1=ALU.add)
        xlp_cm.__exit__(None, None, None)    # free xl/wo_sb

        # ============ Phase 5+6: LN2 + MLP ============
        with tc.tile_pool(name="w1p", bufs=4) as w1p, \
             tc.tile_pool(name="w2p", bufs=2) as w2p, \
             tc.tile_pool(name="tmp2", bufs=2) as tmp2p, \
             tc.tile_pool(name="sgp", bufs=2) as sgp, \
             tc.tile_pool(name="outp", bufs=2) as outp, \
             tc.tile_pool(name="psB", bufs=6, space="PSUM") as psB:
            w1_sb = []
            w2_sb = []
            for g in range(4):   # prefetch during the A2A / Wo phase
                w1t = w1p.tile([128, PO, 1024], BF16, tag="w1")
                nc.sync.dma_start(
                    w1t[:], wf1_in.ap()[:, 1024 * g:1024 * (g + 1)]
                    .rearrange("(po p) n -> p po n", p=128))
                w1_sb.append(w1t)
            for g in range(4):
                w2t = w2p.tile([128, M1, 256], BF16, tag="w2")
                nc.sync.dma_start(
                    w2t[:], wf2_in.ap()[:, 256 * g:256 * (g + 1)]
                    .rearrange("(ko p) n -> p ko n", p=128))
                w2_sb.append(w2t)

            ps_s2 = psB.tile([128, TPC], F32, tag="ps")
            for po in range(PO):
                nc.tensor.matmul(ps_s2[0:1, :], ones_col_r[:],
                                 X2[:, po, :],
                                 start=(po == 0), stop=(po == PO - 1))
            ps_q2 = psB.tile([128, TPC], F32, tag="ps")
            for po in range(PO):
                sq = tmp2p.tile([128, TPC], F32R, tag="sq2")
                nc.vector.tensor_mul(sq[:].bitcast(F32),
                                     X2[:, po, :].bitcast(F32),
                                     X2[:, po, :].bitcast(F32))
                nc.tensor.matmul(ps_q2[0:1, :], ones_col_r[:], sq[:],
                                 start=(po == 0), stop=(po == PO - 1))
            nmu2 = rows.tile([1, TPC], F32R, tag="nmu2")
            nc.scalar.activation(nmu2[:].bitcast(F32), ps_s2[0:1, :],
                                 AF.Copy, scale=-1.0 / D)
            ex2b = rows.tile([1, TPC], F32, tag="ex2b")
            nc.scalar.activation(ex2b[:], ps_q2[0:1, :], AF.Copy,
                                 scale=1.0 / D)
            mu2b = rows.tile([1, TPC], F32, tag="mu2b")
            nc.vector.tensor_mul(mu2b[:], nmu2[:].bitcast(F32),
                                 nmu2[:].bitcast(F32))
            varb = rows.tile([1, TPC], F32, tag="varb")
            nc.vector.scalar_tensor_tensor(
                out=varb[:], in0=ex2b[:], scalar=EPS, in1=mu2b[:],
                op0=ALU.add, op1=ALU.subtract)
            recb = rows.tile([1, TPC], F32, tag="recb")
            nc.vector.reciprocal(recb[:], varb[:])
            inv2 = rows.tile([1, TPC], F32R, tag="inv2")
            nc.scalar.activation(inv2[:].bitcast(F32), recb[:], AF.Sqrt)
            ps_mu = psB.tile([128, TPC], F32, tag="ps")
            nc.tensor.matmul(ps_mu[:], ones_row_r[:], nmu2[:],
                             start=True, stop=True)
            ps_iv = psB.tile([128, TPC], F32, tag="ps")
            nc.tensor.matmul(ps_iv[:], ones_row_r[:], inv2[:],
                             start=True, stop=True)
            for po in range(PO):
                t0 = tmp2p.tile([128, TPC], F32, tag="t0")
                nc.vector.tensor_add(t0[:], X2[:, po, :].bitcast(F32),
                                     ps_mu[:])
                nc.vector.tensor_mul(xh2[:, po, :], t0[:], ps_iv[:])

            for m in range(M1):
                ps = psB.tile([128, TPC], F32, tag="ps")
                w1t = w1_sb[m // 8]
                csl = slice(128 * (m % 8), 128 * (m % 8) + 128)
                for po in range(PO):
                    nc.tensor.matmul(ps[:], w1t[:, po, csl],
                                     xh2[:, po, :],
                                     start=(po == 0),
                                     stop=(po == PO - 1))
                sg = sgp.tile([128, TPC], BF16, tag="sg")
                nc.scalar.activation(sg[:], ps[:], AF.Sigmoid,
                                     scale=SIGC,
                                     bias=bf1s[:, m:m + 1])
                nc.vector.scalar_tensor_tensor(
                    out=Amat[:, m, :], in0=ps[:],
                    scalar=bf1[:, m:m + 1], in1=sg[:],
                    op0=ALU.add, op1=ALU.mult)
            out_view = out_t.ap().rearrange("(po p) t -> p po t", p=128)
            for m in range(PO):
                ps = psB.tile([128, TPC], F32, tag="ps")
                w2t = w2_sb[m // 2]
                csl = slice(128 * (m % 2), 128 * (m % 2) + 128)
                for ko in range(M1):
                    nc.tensor.matmul(ps[:], w2t[:, ko, csl],
                                     Amat[:, ko, :],
                                     start=(ko == 0),
                                     stop=(ko == M1 - 1))
                om = outp.tile([128, TPC], F32, tag="om")
                nc.vector.scalar_tensor_tensor(
                    out=om[:], in0=ps[:], scalar=bf2[:, m:m + 1],
                    in1=X2[:, m, :].bitcast(F32),
                    op0=ALU.add, op1=ALU.add)
                nc.sync.dma_start(out_view[:, m, :], om[:])

    nc.compile()
    return nc


def _get_nc():
    key = ("nc", GELU_NATIVE)
    if key not in _CACHE:
        _CACHE[key] = _build()
    return _CACHE[key]


def _make_in_maps(inputs):
    x = np.asarray(inputs["x"], np.float32).reshape(BT, D)
    ln1w = np.asarray(inputs["ln1_w"], np.float32)
    ln1b = np.asarray(inputs["ln1_b"], np.float32)
    ln2w = np.asarray(inputs["ln2_w"], np.float32)
    ln2b = np.asarray(inputs["ln2_b"], np.float32)
    W_qkv0 = np.asarray(inputs["W_qkv"], np.float32)
    W_qkv = W_qkv0 * ln1w[:, None]
    b_qkv = np.asarray(inputs["b_qkv"], np.float32) + ln1b @ W_qkv0
    W_o = np.asarray(inputs["W_o"], np.float32)
    b_o = np.asarray(inputs["b_o"], np.float32)
    W_ff10 = np.asarray(inputs["W_ff1"], np.float32)
    W_ff1 = W_ff10 * ln2w[:, None]
    b_ff1 = np.asarray(inputs["b_ff1"], np.float32) + ln2b @ W_ff10
    W_ff2 = np.asarray(inputs["W_ff2"], np.float32)
    b_ff2 = np.asarray(inputs["b_ff2"], np.float32)

    def pcol(v):  # [D'] -> [128, D'/128] per-partition column layout
        return np.ascontiguousarray(v.reshape(-1, 128).T)

    xT = np.ascontiguousarray(x.T)                      # [D, BT] f32
    common = {
        "xb": xT.astype(NPBF16),
        "wo": W_o.astype(NPBF16),
        "bo": pcol(b_o),
        "wf1": W_ff1.astype(NPBF16),
        "bf1": pcol(b_ff1),
        "bf1s": pcol(SIGC * b_ff1).astype(np.float32),
        "wf2": W_ff2.astype(NPBF16),
        "bf2": pcol(b_ff2),
    }
    in_maps = []
    for r in range(NC_N):
        hc = 128 * r          # first column of this core's Q/K/V head block
        m = dict(common)
        m["xs"] = np.ascontiguousarray(
            xT[:, TPC * r:TPC * (r + 1)]).astype(NPBF16)
        m["xf"] = np.ascontiguousarray(xT[:, TPC * r:TPC * (r + 1)])
        wq = W_qkv[:, hc:hc + 128]
        wk = W_qkv[:, D + hc:D + hc + 128]
        wv = W_qkv[:, 2 * D + hc:2 * D + hc + 128]
        m["wqkv"] = np.ascontiguousarray(
            np.concatenate([wq, wk, wv], axis=1)).astype(NPBF16)
        m["sqkv"] = np.ascontiguousarray(np.concatenate(
            [wq.sum(0), wk.sum(0)]).reshape(1, 256)).astype(np.float32)
        m["svb"] = np.ascontiguousarray(np.stack(
            [wv.sum(0), b_qkv[2 * D + hc:2 * D + hc + 128]])).astype(NPBF16)
        m["bqkv"] = np.ascontiguousarray(np.stack(
            [b_qkv[hc:hc + 128], b_qkv[D + hc:D + hc + 128]], axis=1))
        in_maps.append(m)
    return in_maps


def _run(inputs, trace=False, trace_cores=None):
    nc = _get_nc()
    in_maps = _make_in_maps(inputs)
    res = bass_utils.run_bass_kernel_spmd(
        nc, in_maps, core_ids=list(range(NC_N)), trace=trace,
        trace_cores=trace_cores)
    outs = [res.results[r]["outt"] for r in range(NC_N)]
    full = np.concatenate([o.T for o in outs], axis=0)
    return full.reshape(B, T, D).astype(np.float32), res


def kernel(**inputs):
    out, _ = _run(inputs, trace=False)
    return out


# revision 30
# speedup vs baseline: 1.9875x; 1.0211x over previous
"""Trainium2 Bass kernel for a dense transformer block (nn_Block_88338887344891).

Distribution over 8 NeuronCores (single SPMD NEFF, 2 collectives):
  - LN1 stats computed per-core on its own 512 tokens, AllGathered (48KB,
    ~16us, overlapped with the QKV matmuls on raw x).
  - LayerNorm is folded into the matmuls: QKV = inv * (W'^T x + s (-mu)) + b'
    where W' = diag(ln_w) W (host-folded), s = colsum(W'), so the heavy
    matmuls start before the stats arrive; ln_b is folded into b'.
  - QKV + causal attention head-sharded (2 heads/core over all 4096 tokens);
    scores/exp/AV trimmed to the causal region; V is produced directly in
    keys-major layout (x as the stationary operand) so no transposes.
  - attention output AllToAll per head in bf16 (1MB, ~41us)
  - output projection + residual + LN2 + full MLP token-sharded (512 tok/core)
  - gelu(tanh approx) via t*sigmoid(1.702 t) (1 ACT + 1 fused DVE op)
All matmul moving operands are bf16 (1 cycle/row); PSUM accumulates f32.
"""
import numpy as np
from contextlib import ExitStack

try:  # persistent XLA cache so repeat runs skip the NEFF compile
    import jax
    jax.config.update("jax_compilation_cache_dir", "/tmp/jax_neff_cache")
    jax.config.update("jax_persistent_cache_min_compile_time_secs", 1.0)
except Exception:
    pass

import ml_dtypes
import concourse.bass as bass
import concourse.bacc as bacc
import concourse.tile as tile
import concourse.mybir as mybir
from concourse import bass_utils

AF = mybir.ActivationFunctionType
ALU = mybir.AluOpType
F32 = mybir.dt.float32
F32R = mybir.dt.float32r
BF16 = mybir.dt.bfloat16
NPBF16 = ml_dtypes.bfloat16

NC_N = 8          # cores
B, T, D, H = 2, 2048, 1024, 16
HD = D // H       # 64
DFF = 4 * D       # 4096
EPS = 1e-5
BT = B * T               # 4096 tokens
TPC = BT // NC_N         # 512 tokens per core
HPC = H // NC_N          # 2 heads per core
PO = D // 128            # 8 D-tiles
M1 = DFF // 128          # 32 ff1 out tiles
NKT = BT // 128          # 32 key tiles globally (16 per batch)
RG = [list(range(NC_N))]
SIGC = 1.702             # gelu sigmoid-form constant

GELU_NATIVE = False   # kept for test.py compat; kernel is sim/hw identical

_CACHE = {}


def _build():
    nc = bacc.Bacc("TRN2", target_bir_lowering=False, debug=False,
                   num_devices=NC_N)

    # ---- per-core external inputs ----
    xb_in = nc.dram_tensor("xb", [D, BT], BF16, kind="ExternalInput")
    xs_in = nc.dram_tensor("xs", [D, TPC], BF16, kind="ExternalInput")
    xf_in = nc.dram_tensor("xf", [D, TPC], F32, kind="ExternalInput")
    wqkv_in = nc.dram_tensor("wqkv", [D, 384], BF16, kind="ExternalInput")
    sqkv_in = nc.dram_tensor("sqkv", [1, 256], F32R, kind="ExternalInput")
    svb_in = nc.dram_tensor("svb", [2, 128], BF16, kind="ExternalInput")
    bqkv_in = nc.dram_tensor("bqkv", [128, 2], F32, kind="ExternalInput")
    wo_in = nc.dram_tensor("wo", [D, D], BF16, kind="ExternalInput")
    bo_in = nc.dram_tensor("bo", [128, PO], F32, kind="ExternalInput")
    wf1_in = nc.dram_tensor("wf1", [D, DFF], BF16, kind="ExternalInput")
    bf1_in = nc.dram_tensor("bf1", [128, M1], F32, kind="ExternalInput")
    bf1s_in = nc.dram_tensor("bf1s", [128, M1], F32, kind="ExternalInput")
    wf2_in = nc.dram_tensor("wf2", [DFF, D], BF16, kind="ExternalInput")
    bf2_in = nc.dram_tensor("bf2", [128, PO], F32, kind="ExternalInput")
    out_t = nc.dram_tensor("outt", [D, TPC], F32, kind="ExternalOutput")

    with tile.TileContext(nc) as tc, ExitStack() as ctx:
        perm = ctx.enter_context(tc.tile_pool(name="perm", bufs=1))
        big = ctx.enter_context(tc.tile_pool(name="big", bufs=1))
        rows = ctx.enter_context(tc.tile_pool(name="rows", bufs=1))
        dram = ctx.enter_context(tc.tile_pool(name="dram", bufs=1, space="DRAM"))

        # ---- constants ----
        ones_col_b = perm.tile([128, 1], BF16)
        nc.vector.memset(ones_col_b[:], 1.0)
        ones_col_f = perm.tile([128, 1], F32)
        nc.vector.memset(ones_col_f[:], 1.0)
        ones_col_r = perm.tile([128, 1], F32R)
        nc.vector.tensor_copy(ones_col_r[:], ones_col_f[:])
        ones_row_f = perm.tile([1, 128], F32)
        nc.vector.memset(ones_row_f[:], 1.0)
        ones_row_r = perm.tile([1, 128], F32R)
        nc.vector.tensor_copy(ones_row_r[:], ones_row_f[:])
        ones_row_b = perm.tile([1, 128], BF16)
        nc.vector.tensor_copy(ones_row_b[:], ones_row_f[:])

        def load_const(t_in, shape, tag, dt=F32):
            t = perm.tile(shape, dt, tag=tag)
            nc.sync.dma_start(t[:], t_in.ap())
            return t

        sqkv = load_const(sqkv_in, [1, 256], "c_sqkv", F32R)
        sv_row = perm.tile([1, 128], BF16, tag="c_sv")
        nc.sync.dma_start(sv_row[:], svb_in.ap()[0:1])
        bv_row = perm.tile([1, 128], BF16, tag="c_bv")
        nc.sync.dma_start(bv_row[:], svb_in.ap()[1:2])
        bqkv = load_const(bqkv_in, [128, 2], "c_bqkv")
        bo = load_const(bo_in, [128, PO], "c_bo")
        bf1 = load_const(bf1_in, [128, M1], "c_bf1")
        bf1s = load_const(bf1s_in, [128, M1], "c_bf1s")
        bf2 = load_const(bf2_in, [128, PO], "c_bf2")

        # ---- persistent SBUF ----
        invc = big.tile([128, NKT], F32R, tag="invc")  # inv keys-major
        mucol = big.tile([128, NKT], F32R, tag="mucol")  # -mu keys-major
        X2 = big.tile([128, PO, TPC], F32R, tag="x2")
        xh2 = big.tile([128, PO, TPC], BF16, tag="xh2")
        Amat = big.tile([128, M1, TPC], BF16, tag="amat")

        # dram scratch
        st_out = dram.tile([2, TPC], F32R)
        st_agg = dram.tile([NC_N, 2, TPC], F32R, addr_space="Shared")
        a2ai = dram.tile([NC_N, 128, TPC], BF16)
        a2ao = dram.tile([NC_N, 128, TPC], BF16)

        xb_view = xb_in.ap().rearrange("(po p) (k t) -> k p po t",
                                       p=128, t=TPC)

        xlp_cm = tc.tile_pool(name="xlp", bufs=1)
        xlp = xlp_cm.__enter__()
        xl = xlp.tile([128, PO, TPC], F32, tag="xl")   # my x (residual)
        wo_sb = xlp.tile([128, PO, D], BF16, tag="wo")
        qkvp_cm = tc.tile_pool(name="qkvp", bufs=1)
        qkvp = qkvp_cm.__enter__()
        Qh = qkvp.tile([128, BT], BF16, tag="qh")      # 2 heads stacked
        Kh = qkvp.tile([128, BT], BF16, tag="kh")
        Vt = qkvp.tile([128, HPC, NKT, 65], BF16, tag="vt")  # keys-major V
        nc.gpsimd.memset(Vt[:, :, :, 64:65], 1.0)   # softmax denominator row

        # ============ Phase 1: local LN1 stats + AllGather ============
        with tc.tile_pool(name="wq", bufs=1) as wq, \
             tc.tile_pool(name="xsp", bufs=1) as xsp, \
             tc.tile_pool(name="xcp", bufs=2) as xcp, \
             tc.tile_pool(name="sqp", bufs=2) as sqp, \
             tc.tile_pool(name="strp", bufs=2) as strp, \
             tc.tile_pool(name="psA", bufs=6, space="PSUM") as psA, \
             tc.tile_pool(name="psV", bufs=2, space="PSUM") as psVt:
            wqkv_sb = wq.tile([128, PO, 384], BF16)
            nc.sync.dma_start(
                wqkv_sb[:], wqkv_in.ap().rearrange("(po p) m -> p po m", p=128))
            xst = xsp.tile([128, PO, TPC], BF16)
            nc.sync.dma_start(
                xst[:], xs_in.ap().rearrange("(po p) t -> p po t", p=128))

            # stats on my 512 tokens
            stmu_l = rows.tile([1, TPC], F32R, tag="stmu_l")
            stinv_l = rows.tile([1, TPC], F32R, tag="stinv_l")
            ps_s = psA.tile([128, TPC], F32, tag="ps")
            for po in range(PO):
                nc.tensor.matmul(ps_s[0:1, :], ones_col_b[:], xst[:, po, :],
                                 start=(po == 0), stop=(po == PO - 1))
            ps_q = psA.tile([128, TPC], F32, tag="ps")
            for po in range(PO):
                sq = sqp.tile([128, TPC], BF16, tag="sq")
                nc.vector.tensor_mul(sq[:], xst[:, po, :], xst[:, po, :])
                nc.tensor.matmul(ps_q[0:1, :], ones_col_b[:], sq[:],
                                 start=(po == 0), stop=(po == PO - 1))
            nc.scalar.activation(stmu_l[:].bitcast(F32), ps_s[0:1, :],
                                 AF.Copy, scale=-1.0 / D)     # -mu
            ex2 = rows.tile([1, TPC], F32, tag="ex2")
            nc.scalar.activation(ex2[:], ps_q[0:1, :], AF.Copy, scale=1.0 / D)
            mu2 = rows.tile([1, TPC], F32, tag="mu2")
            nc.vector.tensor_mul(mu2[:], stmu_l[:].bitcast(F32),
                                 stmu_l[:].bitcast(F32))
            var = rows.tile([1, TPC], F32, tag="var")
            nc.vector.scalar_tensor_tensor(
                out=var[:], in0=ex2[:], scalar=EPS, in1=mu2[:],
                op0=ALU.add, op1=ALU.subtract)
            rec = rows.tile([1, TPC], F32, tag="rec")
            nc.vector.reciprocal(rec[:], var[:])
            nc.scalar.activation(stinv_l[:].bitcast(F32), rec[:],
                                 AF.Sqrt)                      # inv = 1/std
            nc.sync.dma_start(st_out[0:1, :], stmu_l[:])
            nc.sync.dma_start(st_out[1:2, :], stinv_l[:])
            nc.gpsimd.collective_compute(
                "AllGather", ALU.bypass, replica_groups=RG,
                ins=[st_out[:].opt()], outs=[st_agg[:].opt()])
            # gathered stats (ACT dma queue: don't block SP chunk stream)
            st_agg_v = st_agg[:].rearrange("s r t -> r s t")
            # s_v / b_v broadcast along partitions (stats-independent)
            ps_sv = psVt.tile([128, 128], F32, tag="psv")
            nc.tensor.matmul(ps_sv[:], ones_row_b[:], sv_row[:],
                             start=True, stop=True)
            s_vB = perm.tile([128, 128], F32, tag="svB")
            nc.scalar.activation(s_vB[:], ps_sv[:], AF.Copy)
            ps_bv = psVt.tile([128, 128], F32, tag="psv")
            nc.tensor.matmul(ps_bv[:], ones_row_b[:], bv_row[:],
                             start=True, stop=True)
            b_vB = perm.tile([128, 128], F32, tag="bvB")
            nc.scalar.activation(b_vB[:], ps_bv[:], AF.Copy)

            # ===== Phase 2A: raw QKV matmuls (no stats dependency) =====
            for c in range(NC_N):
                xc = xcp.tile([128, PO, TPC], BF16, tag="xc")
                nc.sync.dma_start(xc[:], xb_view[c])
                tok = slice(TPC * c, TPC * (c + 1))
                for m, DST in ((0, Qh), (1, Kh)):
                    ps = psA.tile([128, TPC], F32, tag="ps")
                    for po in range(PO):
                        nc.tensor.matmul(
                            ps[:], wqkv_sb[:, po, 128 * m:128 * m + 128],
                            xc[:, po, :], start=(po == 0), stop=(po == PO - 1))
                    nc.scalar.activation(DST[:, tok], ps[:], AF.Copy)
                for kt in range(4):   # V in keys-major layout
                    g = 4 * c + kt
                    ksl = slice(128 * kt, 128 * kt + 128)
                    psv = psVt.tile([128, 128], F32, tag="psv")
                    for po in range(PO):
                        nc.tensor.matmul(
                            psv[:], xc[:, po, ksl], wqkv_sb[:, po, 256:384],
                            start=(po == 0), stop=(po == PO - 1))
                    for h in range(HPC):
                        nc.scalar.activation(
                            Vt[:, h, g, 0:64], psv[:, 64 * h:64 * h + 64],
                            AF.Copy)

            # residual x + Wo weights: needed from phase 4 on
            nc.sync.dma_start(xl[:],
                              xf_in.ap().rearrange("(po p) t -> p po t", p=128))
            nc.sync.dma_start(
                wo_sb[:], wo_in.ap().rearrange("(po p) n -> p po n", p=128))

            # ===== Phase 2B: fold LN stats in (after AllGather lands) =====
            # inv/-mu keys-major: invc[p, g] = inv[128*g + p]
            st_agg_c = st_agg[:].rearrange("s r (c p) -> s r p c", p=128)
            for s in range(NC_N):
                nc.scalar.dma_start(invc[:, 4 * s:4 * s + 4], st_agg_c[s, 1])
                nc.scalar.dma_start(mucol[:, 4 * s:4 * s + 4], st_agg_c[s, 0])
            for c in range(NC_N):
                tok = slice(TPC * c, TPC * (c + 1))
                murow = strp.tile([1, TPC], F32R, tag="mur")
                nc.scalar.dma_start(murow[:], st_agg_v[0:1, c])
                invrow = strp.tile([1, TPC], F32R, tag="ivr")
                nc.scalar.dma_start(invrow[:], st_agg_v[1:2, c])
                wrow = strp.tile([1, TPC], F32R, tag="wrow")
                nc.vector.tensor_mul(wrow[:].bitcast(F32),
                                     murow[:].bitcast(F32),
                                     invrow[:].bitcast(F32))
                psb = psA.tile([128, TPC], F32, tag="ps")
                nc.tensor.matmul(psb[:], ones_row_r[:], invrow[:],
                                 start=True, stop=True)
                for m, DST in ((0, Qh), (1, Kh)):
                    mc = psA.tile([128, TPC], F32, tag="ps")
                    nc.tensor.matmul(mc[:], sqkv[:, 128 * m:128 * m + 128],
                                     wrow[:], start=True, stop=True)
                    tmp = sqp.tile([128, TPC], F32, tag="tmpqk")
                    nc.vector.tensor_mul(tmp[:], DST[:, tok], psb[:])
                    nc.vector.scalar_tensor_tensor(
                        out=DST[:, tok], in0=mc[:], scalar=bqkv[:, m:m + 1],
                        in1=tmp[:], op0=ALU.add, op1=ALU.add)
                for kt in range(4):
                    g = 4 * c + kt
                    for h in range(HPC):
                        hs = slice(64 * h, 64 * h + 64)
                        vtmp = sqp.tile([128, 64], F32, tag="vtmp")
                        nc.vector.scalar_tensor_tensor(
                            out=vtmp[:], in0=s_vB[:, hs],
                            scalar=mucol[:, g:g + 1].bitcast(F32),
                            in1=Vt[:, h, g, 0:64],
                            op0=ALU.mult, op1=ALU.add)
                        nc.vector.scalar_tensor_tensor(
                            out=Vt[:, h, g, 0:64], in0=vtmp[:],
                            scalar=invc[:, g:g + 1].bitcast(F32),
                            in1=b_vB[:, hs], op0=ALU.mult, op1=ALU.add)

        # ============ Phase 3: causal attention ============
        with tc.tile_pool(name="ptp", bufs=3) as ptp, \
             tc.tile_pool(name="avp", bufs=2) as avp, \
             tc.tile_pool(name="psS", bufs=2, space="PSUM") as psS, \
             tc.tile_pool(name="psV2", bufs=2, space="PSUM") as psV2:
            for b in range(B):
                for h in range(HPC):
                    hsl = slice(64 * h, 64 * h + 64)
                    for qh in range(2):
                        n_i = 8 if qh == 0 else 16
                        ps_av = psV2.tile([65, 1024], F32, tag="psav")
                        for i in range(n_i):
                            koff = 2048 * b + 128 * i
                            qlo = max(1024 * qh, 128 * i)
                            free = 1024 * (qh + 1) - qlo
                            off = qlo - 1024 * qh
                            qabs = 2048 * b + qlo
                            # matmul outputs must stay inside one psum bank
                            # (512 f32): emit per-bank-half pieces, aligned
                            # to each psum tile's own banks.
                            pieces = []   # ps_av-aligned (offset off..1024)
                            lo = off
                            while lo < 1024:
                                hi = min(1024, (lo // 512 + 1) * 512)
                                pieces.append((lo, hi - lo))
                                lo = hi
                            spieces = []  # pss-aligned (offset 0..free)
                            lo = 0
                            while lo < free:
                                hi = min(free, (lo // 512 + 1) * 512)
                                spieces.append((lo, hi - lo))
                                lo = hi
                            pss = psS.tile([128, 1024], F32, tag="pss")
                            for (plo, pfree) in spieces:
                                qa = qabs + plo
                                nc.tensor.matmul(
                                    pss[:, plo:plo + pfree],
                                    Kh[hsl, koff:koff + 128],
                                    Qh[hsl, qa:qa + pfree],
                                    start=True, stop=True)
                            pt = ptp.tile([128, 1024], BF16, tag="pt")
                            nc.scalar.activation(pt[:, 0:free],
                                                 pss[:, 0:free],
                                                 AF.Exp, scale=0.125)
                            if 128 * i >= 1024 * qh:   # diagonal tile
                                nc.gpsimd.affine_select(
                                    out=pt[:, 0:128], in_=pt[:, 0:128],
                                    compare_op=ALU.is_ge, fill=0.0,
                                    base=0, pattern=[[1, 128]],
                                    channel_multiplier=-1)
                            for (plo, pfree) in pieces:
                                hb = plo // 512
                                last_i = 8 * qh + 3 if hb == 0 else n_i - 1
                                nc.tensor.matmul(
                                    ps_av[:, plo:plo + pfree],
                                    Vt[:, h, 16 * b + i, :],
                                    pt[:, plo - off:plo - off + pfree],
                                    start=(i == 0), stop=(i == last_i))
                        recd = rows.tile([1, 1024], F32R, tag="recd")
                        nc.vector.reciprocal(recd[:].bitcast(F32),
                                             ps_av[64:65, :])
                        ps_bc = psS.tile([128, 1024], F32, tag="pss")
                        for half in range(2):
                            nc.tensor.matmul(
                                ps_bc[0:64, 512 * half:512 * half + 512],
                                ones_row_r[:, 0:64],
                                recd[:, 512 * half:512 * half + 512],
                                start=True, stop=True)
                        avn = avp.tile([64, 1024], BF16, tag="avn")
                        nc.vector.tensor_mul(avn[:], ps_av[0:64, :],
                                             ps_bc[0:64, :])
                        g0 = 4 * b + 2 * qh
                        nc.sync.dma_start(a2ai[g0, hsl, :], avn[:, 0:TPC])
                        nc.sync.dma_start(a2ai[g0 + 1, hsl, :],
                                          avn[:, TPC:2 * TPC])

        nc.gpsimd.collective_compute(
            "AllToAll", ALU.bypass, replica_groups=RG,
            ins=[a2ai[:].opt()], outs=[a2ao[:].opt()])
        qkvp_cm.__exit__(None, None, None)   # free Qh/Kh/Vt/invB

        # ============ Phase 4: output projection + residual ============
        with tc.tile_pool(name="avtp", bufs=1) as avtp, \
             tc.tile_pool(name="psA2", bufs=6, space="PSUM") as psA2:
            AVt = avtp.tile([128, NC_N, TPC], BF16)
            nc.scalar.dma_start(AVt[:], a2ao[:].rearrange("s p t -> p s t"))
            for m in range(PO):
                ps = psA2.tile([128, TPC], F32, tag="ps")
                for po in range(PO):
                    nc.tensor.matmul(
                        ps[:], wo_sb[:, po, 128 * m:128 * m + 128],
                        AVt[:, po, :], start=(po == 0), stop=(po == PO - 1))
                nc.vector.scalar_tensor_tensor(
                    out=X2[:, m, :].bitcast(F32), in0=ps[:],
                    scalar=bo[:, m:m + 1], in1=xl[:, m, :],
                    op0=ALU.add, op1=ALU.add)
        xlp_cm.__exit__(None, None, None)    # free xl/wo_sb

        # ============ Phase 5+6: LN2 + MLP ============
        with tc.tile_pool(name="w1p", bufs=4) as w1p, \
             tc.tile_pool(name="w2p", bufs=2) as w2p, \
             tc.tile_pool(name="tmp2", bufs=2) as tmp2p, \
             tc.tile_pool(name="sgp", bufs=2) as sgp, \
             tc.tile_pool(name="outp", bufs=2) as outp, \
             tc.tile_pool(name="psB", bufs=6, space="PSUM") as psB:
            w1_sb = []
            w2_sb = []
            for g in range(4):   # prefetch during the A2A / Wo phase
                w1t = w1p.tile([128, PO, 1024], BF16, tag="w1")
                nc.sync.dma_start(
                    w1t[:], wf1_in.ap()[:, 1024 * g:1024 * (g + 1)]
                    .rearrange("(po p) n -> p po n", p=128))
                w1_sb.append(w1t)
            for g in range(4):
                w2t = w2p.tile([128, M1, 256], BF16, tag="w2")
                nc.sync.dma_start(
                    w2t[:], wf2_in.ap()[:, 256 * g:256 * (g + 1)]
                    .rearrange("(ko p) n -> p ko n", p=128))
                w2_sb.append(w2t)

            ps_s2 = psB.tile([128, TPC], F32, tag="ps")
            for po in range(PO):
                nc.tensor.matmul(ps_s2[0:1, :], ones_col_r[:],
                                 X2[:, po, :],
                                 start=(po == 0), stop=(po == PO - 1))
            ps_q2 = psB.tile([128, TPC], F32, tag="ps")
            for po in range(PO):
                sq = tmp2p.tile([128, TPC], F32R, tag="sq2")
                nc.vector.tensor_mul(sq[:].bitcast(F32),
                                     X2[:, po, :].bitcast(F32),
                                     X2[:, po, :].bitcast(F32))
                nc.tensor.matmul(ps_q2[0:1, :], ones_col_r[:], sq[:],
                                 start=(po == 0), stop=(po == PO - 1))
            nmu2 = rows.tile([1, TPC], F32R, tag="nmu2")
            nc.scalar.activation(nmu2[:].bitcast(F32), ps_s2[0:1, :],
                                 AF.Copy, scale=-1.0 / D)
            ex2b = rows.tile([1, TPC], F32, tag="ex2b")
            nc.scalar.activation(ex2b[:], ps_q2[0:1, :], AF.Copy,
                                 scale=1.0 / D)
            mu2b = rows.tile([1, TPC], F32, tag="mu2b")
            nc.vector.tensor_mul(mu2b[:], nmu2[:].bitcast(F32),
                                 nmu2[:].bitcast(F32))
            varb = rows.tile([1, TPC], F32, tag="varb")
            nc.vector.scalar_tensor_tensor(
                out=varb[:], in0=ex2b[:], scalar=EPS, in1=mu2b[:],
                op0=ALU.add, op1=ALU.subtract)
            recb = rows.tile([1, TPC], F32, tag="recb")
            nc.vector.reciprocal(recb[:], varb[:])
            inv2 = rows.tile([1, TPC], F32R, tag="inv2")
            nc.scalar.activation(inv2[:].bitcast(F32), recb[:], AF.Sqrt)
            ps_mu = psB.tile([128, TPC], F32, tag="ps")
            nc.tensor.matmul(ps_mu[:], ones_row_r[:], nmu2[:],
                             start=True, stop=True)
            ps_iv = psB.tile([128, TPC], F32, tag="ps")
            nc.tensor.matmul(ps_iv[:], ones_row_r[:], inv2[:],
                             start=True, stop=True)
            for po in range(PO):
                t0 = tmp2p.tile([128, TPC], F32, tag="t0")
                nc.vector.tensor_add(t0[:], X2[:, po, :].bitcast(F32),
                                     ps_mu[:])
                nc.vector.tensor_mul(xh2[:, po, :], t0[:], ps_iv[:])

            for m in range(M1):
                ps = psB.tile([128, TPC], F32, tag="ps")
                w1t = w1_sb[m // 8]
                csl = slice(128 * (m % 8), 128 * (m % 8) + 128)
                for po in range(PO):
                    nc.tensor.matmul(ps[:], w1t[:, po, csl],
                                     xh2[:, po, :],
                                     start=(po == 0),
                                     stop=(po == PO - 1))
                sg = sgp.tile([128, TPC], BF16, tag="sg")
                nc.scalar.activation(sg[:], ps[:], AF.Sigmoid,
                                     scale=SIGC,
                                     bias=bf1s[:, m:m + 1])
                nc.vector.scalar_tensor_tensor(
                    out=Amat[:, m, :], in0=ps[:],
                    scalar=bf1[:, m:m + 1], in1=sg[:],
                    op0=ALU.add, op1=ALU.mult)
            out_view = out_t.ap().rearrange("(po p) t -> p po t", p=128)
            for m in range(PO):
                ps = psB.tile([128, TPC], F32, tag="ps")
                w2t = w2_sb[m // 2]
                csl = slice(128 * (m % 2), 128 * (m % 2) + 128)
                for ko in range(M1):
                    nc.tensor.matmul(ps[:], w2t[:, ko, csl],
                                     Amat[:, ko, :],
                                     start=(ko == 0),
                                     stop=(ko == M1 - 1))
                om = outp.tile([128, TPC], F32, tag="om")
                nc.vector.scalar_tensor_tensor(
                    out=om[:], in0=ps[:], scalar=bf2[:, m:m + 1],
                    in1=X2[:, m, :].bitcast(F32),
                    op0=ALU.add, op1=ALU.add)
                nc.sync.dma_start(out_view[:, m, :], om[:])

    nc.compile()
    return nc


def _get_nc():
    key = ("nc", GELU_NATIVE)
    if key not in _CACHE:
        _CACHE[key] = _build()
    return _CACHE[key]


def _make_in_maps(inputs):
    x = np.asarray(inputs["x"], np.float32).reshape(BT, D)
    ln1w = np.asarray(inputs["ln1_w"], np.float32)
    ln1b = np.asarray(inputs["ln1_b"], np.float32)
    ln2w = np.asarray(inputs["ln2_w"], np.float32)
    ln2b = np.asarray(inputs["ln2_b"], np.float32)
    W_qkv0 = np.asarray(inputs["W_qkv"], np.float32)
    W_qkv = W_qkv0 * ln1w[:, None]
    b_qkv = np.asarray(inputs["b_qkv"], np.float32) + ln1b @ W_qkv0
    W_o = np.asarray(inputs["W_o"], np.float32)
    b_o = np.asarray(inputs["b_o"], np.float32)
    W_ff10 = np.asarray(inputs["W_ff1"], np.float32)
    W_ff1 = W_ff10 * ln2w[:, None]
    b_ff1 = np.asarray(inputs["b_ff1"], np.float32) + ln2b @ W_ff10
    W_ff2 = np.asarray(inputs["W_ff2"], np.float32)
    b_ff2 = np.asarray(inputs["b_ff2"], np.float32)

    def pcol(v):  # [D'] -> [128, D'/128] per-partition column layout
        return np.ascontiguousarray(v.reshape(-1, 128).T)

    xT = np.ascontiguousarray(x.T)                      # [D, BT] f32
    common = {
        "xb": xT.astype(NPBF16),
        "wo": W_o.astype(NPBF16),
        "bo": pcol(b_o),
        "wf1": W_ff1.astype(NPBF16),
        "bf1": pcol(b_ff1),
        "bf1s": pcol(SIGC * b_ff1).astype(np.float32),
        "wf2": W_ff2.astype(NPBF16),
        "bf2": pcol(b_ff2),
    }
    in_maps = []
    for r in range(NC_N):
        hc = 128 * r          # first column of this core's Q/K/V head block
        m = dict(common)
        m["xs"] = np.ascontiguousarray(
            xT[:, TPC * r:TPC * (r + 1)]).astype(NPBF16)
        m["xf"] = np.ascontiguousarray(xT[:, TPC * r:TPC * (r + 1)])
        wq = W_qkv[:, hc:hc + 128]
        wk = W_qkv[:, D + hc:D + hc + 128]
        wv = W_qkv[:, 2 * D + hc:2 * D + hc + 128]
        m["wqkv"] = np.ascontiguousarray(
            np.concatenate([wq, wk, wv], axis=1)).astype(NPBF16)
        m["sqkv"] = np.ascontiguousarray(np.concatenate(
            [wq.sum(0), wk.sum(0)]).reshape(1, 256)).astype(np.float32)
        m["svb"] = np.ascontiguousarray(np.stack(
            [wv.sum(0), b_qkv[2 * D + hc:2 * D + hc + 128]])).astype(NPBF16)
        m["bqkv"] = np.ascontiguousarray(np.stack(
            [b_qkv[hc:hc + 128], b_qkv[D + hc:D + hc + 128]], axis=1))
        in_maps.append(m)
    return in_maps


def _run(inputs, trace=False, trace_cores=None):
    nc = _get_nc()
    in_maps = _make_in_maps(inputs)
    res = bass_utils.run_bass_kernel_spmd(
        nc, in_maps, core_ids=list(range(NC_N)), trace=trace,
        trace_cores=trace_cores)
    outs = [res.results[r]["outt"] for r in range(NC_N)]
    full = np.concatenate([o.T for o in outs], axis=0)
    return full.reshape(B, T, D).astype(np.float32), res


def kernel(**inputs):
    out, _ = _run(inputs, trace=False)
    return out


# revision 31
# speedup vs baseline: 2.0379x; 1.0254x over previous
"""Trainium2 Bass kernel for a dense transformer block (nn_Block_88338887344891).

Distribution over 8 NeuronCores (single SPMD NEFF, 2 collectives):
  - LN1 stats computed per-core on its own 512 tokens, AllGathered (48KB,
    ~16us, overlapped with the QKV matmuls on raw x).
  - LayerNorm is folded into the matmuls: QKV = inv * (W'^T x + s (-mu)) + b'
    where W' = diag(ln_w) W (host-folded), s = colsum(W'), so the heavy
    matmuls start before the stats arrive; ln_b is folded into b'.
  - QKV + causal attention head-sharded (2 heads/core over all 4096 tokens);
    scores/exp/AV trimmed to the causal region; V is produced directly in
    keys-major layout (x as the stationary operand) so no transposes.
  - attention output AllToAll per head in bf16 (1MB, ~41us)
  - output projection + residual + LN2 + full MLP token-sharded (512 tok/core)
  - gelu(tanh approx) via t*sigmoid(1.702 t) (1 ACT + 1 fused DVE op)
All matmul moving operands are bf16 (1 cycle/row); PSUM accumulates f32.
"""
import numpy as np
from contextlib import ExitStack

try:  # persistent XLA cache so repeat runs skip the NEFF compile
    import jax
    jax.config.update("jax_compilation_cache_dir", "/tmp/jax_neff_cache")
    jax.config.update("jax_persistent_cache_min_compile_time_secs", 1.0)
except Exception:
    pass

import ml_dtypes
import concourse.bass as bass
import concourse.bacc as bacc
import concourse.tile as tile
import concourse.mybir as mybir
from concourse import bass_utils

AF = mybir.ActivationFunctionType
ALU = mybir.AluOpType
F32 = mybir.dt.float32
F32R = mybir.dt.float32r
BF16 = mybir.dt.bfloat16
NPBF16 = ml_dtypes.bfloat16

NC_N = 8          # cores
B, T, D, H = 2, 2048, 1024, 16
HD = D // H       # 64
DFF = 4 * D       # 4096
EPS = 1e-5
BT = B * T               # 4096 tokens
TPC = BT // NC_N         # 512 tokens per core
HPC = H // NC_N          # 2 heads per core
PO = D // 128            # 8 D-tiles
M1 = DFF // 128          # 32 ff1 out tiles
NKT = BT // 128          # 32 key tiles globally (16 per batch)
RG = [list(range(NC_N))]
SIGC = 1.702             # gelu sigmoid-form constant

GELU_NATIVE = False   # kept for test.py compat; kernel is sim/hw identical

_CACHE = {}


def _build():
    nc = bacc.Bacc("TRN2", target_bir_lowering=False, debug=False,
                   num_devices=NC_N)

    # ---- per-core external inputs ----
    xb_in = nc.dram_tensor("xb", [D, BT], BF16, kind="ExternalInput")
    xs_in = nc.dram_tensor("xs", [D, TPC], BF16, kind="ExternalInput")
    xf_in = nc.dram_tensor("xf", [D, TPC], F32, kind="ExternalInput")
    wqkv_in = nc.dram_tensor("wqkv", [D, 384], BF16, kind="ExternalInput")
    sqkv_in = nc.dram_tensor("sqkv", [1, 256], F32R, kind="ExternalInput")
    svb_in = nc.dram_tensor("svb", [2, 128], BF16, kind="ExternalInput")
    bqkv_in = nc.dram_tensor("bqkv", [128, 2], F32, kind="ExternalInput")
    wo_in = nc.dram_tensor("wo", [D, D], BF16, kind="ExternalInput")
    bo_in = nc.dram_tensor("bo", [128, PO], F32, kind="ExternalInput")
    wf1_in = nc.dram_tensor("wf1", [D, DFF], BF16, kind="ExternalInput")
    bf1_in = nc.dram_tensor("bf1", [128, M1], F32, kind="ExternalInput")
    bf1s_in = nc.dram_tensor("bf1s", [128, M1], F32, kind="ExternalInput")
    wf2_in = nc.dram_tensor("wf2", [DFF, D], BF16, kind="ExternalInput")
    bf2_in = nc.dram_tensor("bf2", [128, PO], F32, kind="ExternalInput")
    out_t = nc.dram_tensor("outt", [D, TPC], F32, kind="ExternalOutput")

    with tile.TileContext(nc) as tc, ExitStack() as ctx:
        perm = ctx.enter_context(tc.tile_pool(name="perm", bufs=1))
        big = ctx.enter_context(tc.tile_pool(name="big", bufs=1))
        rows = ctx.enter_context(tc.tile_pool(name="rows", bufs=1))
        dram = ctx.enter_context(tc.tile_pool(name="dram", bufs=1, space="DRAM"))

        # ---- constants ----
        ones_col_b = perm.tile([128, 1], BF16)
        nc.vector.memset(ones_col_b[:], 1.0)
        ones_col_f = perm.tile([128, 1], F32)
        nc.vector.memset(ones_col_f[:], 1.0)
        ones_col_r = perm.tile([128, 1], F32R)
        nc.vector.tensor_copy(ones_col_r[:], ones_col_f[:])
        ones_row_f = perm.tile([1, 128], F32)
        nc.vector.memset(ones_row_f[:], 1.0)
        ones_row_r = perm.tile([1, 128], F32R)
        nc.vector.tensor_copy(ones_row_r[:], ones_row_f[:])
        ones_row_b = perm.tile([1, 128], BF16)
        nc.vector.tensor_copy(ones_row_b[:], ones_row_f[:])

        def load_const(t_in, shape, tag, dt=F32):
            t = perm.tile(shape, dt, tag=tag)
            nc.sync.dma_start(t[:], t_in.ap())
            return t

        sqkv = load_const(sqkv_in, [1, 256], "c_sqkv", F32R)
        sv_row = perm.tile([1, 128], BF16, tag="c_sv")
        nc.sync.dma_start(sv_row[:], svb_in.ap()[0:1])
        bv_row = perm.tile([1, 128], BF16, tag="c_bv")
        nc.sync.dma_start(bv_row[:], svb_in.ap()[1:2])
        bqkv = load_const(bqkv_in, [128, 2], "c_bqkv")
        bo = load_const(bo_in, [128, PO], "c_bo")
        bf1 = load_const(bf1_in, [128, M1], "c_bf1")
        bf1s = load_const(bf1s_in, [128, M1], "c_bf1s")
        bf2 = load_const(bf2_in, [128, PO], "c_bf2")

        # ---- persistent SBUF ----
        invc = big.tile([128, NKT], F32R, tag="invc")  # inv keys-major
        mucol = big.tile([128, NKT], F32R, tag="mucol")  # -mu keys-major
        X2 = big.tile([128, PO, TPC], F32R, tag="x2")
        xh2 = big.tile([128, PO, TPC], BF16, tag="xh2")
        Amat = big.tile([128, M1, TPC], BF16, tag="amat")

        # dram scratch
        st_out = dram.tile([2, TPC], F32R)
        st_agg = dram.tile([NC_N, 2, TPC], F32R, addr_space="Shared")
        a2ai = dram.tile([NC_N, 128, TPC], BF16)
        a2ao = dram.tile([NC_N, 128, TPC], BF16)

        xb_view = xb_in.ap().rearrange("(po p) (k t) -> k p po t",
                                       p=128, t=TPC)

        xlp_cm = tc.tile_pool(name="xlp", bufs=1)
        xlp = xlp_cm.__enter__()
        xl = xlp.tile([128, PO, TPC], F32, tag="xl")   # my x (residual)
        wo_sb = xlp.tile([128, PO, D], BF16, tag="wo")
        qkvp_cm = tc.tile_pool(name="qkvp", bufs=1)
        qkvp = qkvp_cm.__enter__()
        Qh = qkvp.tile([128, BT], BF16, tag="qh")      # 2 heads stacked
        Kh = qkvp.tile([128, BT], BF16, tag="kh")
        Vt = qkvp.tile([128, HPC, NKT, 65], BF16, tag="vt")  # keys-major V
        nc.gpsimd.memset(Vt[:, :, :, 64:65], 1.0)   # softmax denominator row

        # ============ Phase 1: local LN1 stats + AllGather ============
        with tc.tile_pool(name="wq", bufs=1) as wq, \
             tc.tile_pool(name="xsp", bufs=1) as xsp, \
             tc.tile_pool(name="xcp", bufs=2) as xcp, \
             tc.tile_pool(name="sqp", bufs=2) as sqp, \
             tc.tile_pool(name="strp", bufs=2) as strp, \
             tc.tile_pool(name="psA", bufs=6, space="PSUM") as psA, \
             tc.tile_pool(name="psV", bufs=2, space="PSUM") as psVt:
            wqkv_sb = wq.tile([128, PO, 384], BF16)
            nc.sync.dma_start(
                wqkv_sb[:], wqkv_in.ap().rearrange("(po p) m -> p po m", p=128))
            xc0 = xcp.tile([128, PO, TPC], BF16, tag="xc")
            nc.sync.dma_start(xc0[:], xb_view[0])
            xst = xsp.tile([128, PO, TPC], BF16)
            nc.sync.dma_start(
                xst[:], xs_in.ap().rearrange("(po p) t -> p po t", p=128))

            # stats on my 512 tokens
            stmu_l = rows.tile([1, TPC], F32R, tag="stmu_l")
            stinv_l = rows.tile([1, TPC], F32R, tag="stinv_l")
            ps_s = psA.tile([128, TPC], F32, tag="ps")
            for po in range(PO):
                nc.tensor.matmul(ps_s[0:1, :], ones_col_b[:], xst[:, po, :],
                                 start=(po == 0), stop=(po == PO - 1))
            ps_q = psA.tile([128, TPC], F32, tag="ps")
            for po in range(PO):
                sq = sqp.tile([128, TPC], BF16, tag="sq")
                nc.vector.tensor_mul(sq[:], xst[:, po, :], xst[:, po, :])
                nc.tensor.matmul(ps_q[0:1, :], ones_col_b[:], sq[:],
                                 start=(po == 0), stop=(po == PO - 1))
            nc.scalar.activation(stmu_l[:].bitcast(F32), ps_s[0:1, :],
                                 AF.Copy, scale=-1.0 / D)     # -mu
            ex2 = rows.tile([1, TPC], F32, tag="ex2")
            nc.scalar.activation(ex2[:], ps_q[0:1, :], AF.Copy, scale=1.0 / D)
            mu2 = rows.tile([1, TPC], F32, tag="mu2")
            nc.vector.tensor_mul(mu2[:], stmu_l[:].bitcast(F32),
                                 stmu_l[:].bitcast(F32))
            var = rows.tile([1, TPC], F32, tag="var")
            nc.vector.scalar_tensor_tensor(
                out=var[:], in0=ex2[:], scalar=EPS, in1=mu2[:],
                op0=ALU.add, op1=ALU.subtract)
            rec = rows.tile([1, TPC], F32, tag="rec")
            nc.vector.reciprocal(rec[:], var[:])
            nc.scalar.activation(stinv_l[:].bitcast(F32), rec[:],
                                 AF.Sqrt)                      # inv = 1/std
            nc.sync.dma_start(st_out[0:1, :], stmu_l[:])
            nc.sync.dma_start(st_out[1:2, :], stinv_l[:])
            nc.gpsimd.collective_compute(
                "AllGather", ALU.bypass, replica_groups=RG,
                ins=[st_out[:].opt()], outs=[st_agg[:].opt()])
            # gathered stats (ACT dma queue: don't block SP chunk stream)
            st_agg_v = st_agg[:].rearrange("s r t -> r s t")
            # s_v / b_v broadcast along partitions (stats-independent)
            ps_sv = psVt.tile([128, 128], F32, tag="psv")
            nc.tensor.matmul(ps_sv[:], ones_row_b[:], sv_row[:],
                             start=True, stop=True)
            s_vB = perm.tile([128, 128], F32, tag="svB")
            nc.scalar.activation(s_vB[:], ps_sv[:], AF.Copy)
            ps_bv = psVt.tile([128, 128], F32, tag="psv")
            nc.tensor.matmul(ps_bv[:], ones_row_b[:], bv_row[:],
                             start=True, stop=True)
            b_vB = perm.tile([128, 128], F32, tag="bvB")
            nc.scalar.activation(b_vB[:], ps_bv[:], AF.Copy)

            # ===== Phase 2A: raw QKV matmuls (no stats dependency) =====
            for c in range(NC_N):
                if c == 0:
                    xc = xc0
                else:
                    xc = xcp.tile([128, PO, TPC], BF16, tag="xc")
                    nc.sync.dma_start(xc[:], xb_view[c])
                tok = slice(TPC * c, TPC * (c + 1))
                for m, DST in ((0, Qh), (1, Kh)):
                    ps = psA.tile([128, TPC], F32, tag="ps")
                    for po in range(PO):
                        nc.tensor.matmul(
                            ps[:], wqkv_sb[:, po, 128 * m:128 * m + 128],
                            xc[:, po, :], start=(po == 0), stop=(po == PO - 1))
                    nc.scalar.activation(DST[:, tok], ps[:], AF.Copy)
                for kt in range(4):   # V in keys-major layout
                    g = 4 * c + kt
                    ksl = slice(128 * kt, 128 * kt + 128)
                    psv = psVt.tile([128, 128], F32, tag="psv")
                    for po in range(PO):
                        nc.tensor.matmul(
                            psv[:], xc[:, po, ksl], wqkv_sb[:, po, 256:384],
                            start=(po == 0), stop=(po == PO - 1))
                    for h in range(HPC):
                        nc.scalar.activation(
                            Vt[:, h, g, 0:64], psv[:, 64 * h:64 * h + 64],
                            AF.Copy)

            # residual x + Wo weights: needed from phase 4 on
            nc.sync.dma_start(xl[:],
                              xf_in.ap().rearrange("(po p) t -> p po t", p=128))
            nc.sync.dma_start(
                wo_sb[:], wo_in.ap().rearrange("(po p) n -> p po n", p=128))

            # ===== Phase 2B: fold LN stats in (after AllGather lands) =====
            # inv/-mu keys-major: invc[p, g] = inv[128*g + p]
            st_agg_c = st_agg[:].rearrange("s r (c p) -> s r p c", p=128)
            for s in range(NC_N):
                nc.scalar.dma_start(invc[:, 4 * s:4 * s + 4], st_agg_c[s, 1])
                nc.scalar.dma_start(mucol[:, 4 * s:4 * s + 4], st_agg_c[s, 0])
            for c in range(NC_N):
                tok = slice(TPC * c, TPC * (c + 1))
                murow = strp.tile([1, TPC], F32R, tag="mur")
                nc.scalar.dma_start(murow[:], st_agg_v[0:1, c])
                invrow = strp.tile([1, TPC], F32R, tag="ivr")
                nc.scalar.dma_start(invrow[:], st_agg_v[1:2, c])
                wrow = strp.tile([1, TPC], F32R, tag="wrow")
                nc.vector.tensor_mul(wrow[:].bitcast(F32),
                                     murow[:].bitcast(F32),
                                     invrow[:].bitcast(F32))
                psb = psA.tile([128, TPC], F32, tag="ps")
                nc.tensor.matmul(psb[:], ones_row_r[:], invrow[:],
                                 start=True, stop=True)
                for m, DST in ((0, Qh), (1, Kh)):
                    mc = psA.tile([128, TPC], F32, tag="ps")
                    nc.tensor.matmul(mc[:], sqkv[:, 128 * m:128 * m + 128],
                                     wrow[:], start=True, stop=True)
                    tmp = sqp.tile([128, TPC], F32, tag="tmpqk")
                    nc.vector.tensor_mul(tmp[:], DST[:, tok], psb[:])
                    nc.vector.scalar_tensor_tensor(
                        out=DST[:, tok], in0=mc[:], scalar=bqkv[:, m:m + 1],
                        in1=tmp[:], op0=ALU.add, op1=ALU.add)
                for kt in range(4):
                    g = 4 * c + kt
                    for h in range(HPC):
                        hs = slice(64 * h, 64 * h + 64)
                        vtmp = sqp.tile([128, 64], F32, tag="vtmp")
                        nc.vector.scalar_tensor_tensor(
                            out=vtmp[:], in0=s_vB[:, hs],
                            scalar=mucol[:, g:g + 1].bitcast(F32),
                            in1=Vt[:, h, g, 0:64],
                            op0=ALU.mult, op1=ALU.add)
                        nc.vector.scalar_tensor_tensor(
                            out=Vt[:, h, g, 0:64], in0=vtmp[:],
                            scalar=invc[:, g:g + 1].bitcast(F32),
                            in1=b_vB[:, hs], op0=ALU.mult, op1=ALU.add)

        # ============ Phase 3: causal attention ============
        with tc.tile_pool(name="ptp", bufs=3) as ptp, \
             tc.tile_pool(name="avp", bufs=2) as avp, \
             tc.tile_pool(name="psS", bufs=2, space="PSUM") as psS, \
             tc.tile_pool(name="psV2", bufs=2, space="PSUM") as psV2:
            pending = []
            for b in range(B):
                for h in range(HPC):
                    hsl = slice(64 * h, 64 * h + 64)
                    for qh in range(2):
                        n_i = 8 if qh == 0 else 16
                        ps_av = psV2.tile([65, 1024], F32, tag="psav")
                        for i in range(n_i):
                            if i == 2 and pending:
                                pending.pop()()
                            koff = 2048 * b + 128 * i
                            qlo = max(1024 * qh, 128 * i)
                            free = 1024 * (qh + 1) - qlo
                            off = qlo - 1024 * qh
                            qabs = 2048 * b + qlo
                            # matmul outputs must stay inside one psum bank
                            # (512 f32): emit per-bank-half pieces, aligned
                            # to each psum tile's own banks.
                            pieces = []   # ps_av-aligned (offset off..1024)
                            lo = off
                            while lo < 1024:
                                hi = min(1024, (lo // 512 + 1) * 512)
                                pieces.append((lo, hi - lo))
                                lo = hi
                            spieces = []  # pss-aligned (offset 0..free)
                            lo = 0
                            while lo < free:
                                hi = min(free, (lo // 512 + 1) * 512)
                                spieces.append((lo, hi - lo))
                                lo = hi
                            pss = psS.tile([128, 1024], F32, tag="pss")
                            for (plo, pfree) in spieces:
                                qa = qabs + plo
                                nc.tensor.matmul(
                                    pss[:, plo:plo + pfree],
                                    Kh[hsl, koff:koff + 128],
                                    Qh[hsl, qa:qa + pfree],
                                    start=True, stop=True)
                            pt = ptp.tile([128, 1024], BF16, tag="pt")
                            nc.scalar.activation(pt[:, 0:free],
                                                 pss[:, 0:free],
                                                 AF.Exp, scale=0.125)
                            if 128 * i >= 1024 * qh:   # diagonal tile
                                nc.gpsimd.affine_select(
                                    out=pt[:, 0:128], in_=pt[:, 0:128],
                                    compare_op=ALU.is_ge, fill=0.0,
                                    base=0, pattern=[[1, 128]],
                                    channel_multiplier=-1)
                            for (plo, pfree) in pieces:
                                hb = plo // 512
                                last_i = 8 * qh + 3 if hb == 0 else n_i - 1
                                nc.tensor.matmul(
                                    ps_av[:, plo:plo + pfree],
                                    Vt[:, h, 16 * b + i, :],
                                    pt[:, plo - off:plo - off + pfree],
                                    start=(i == 0), stop=(i == last_i))
                        # epilogue: recip on DVE now; the PE broadcast is
                        # deferred into the next group's loop so it doesn't
                        # head-of-line block the next scores matmuls.
                        recd = rows.tile([1, 1024], F32R, tag="recd")
                        nc.vector.reciprocal(recd[:].bitcast(F32),
                                             ps_av[64:65, :])

                        def epilogue(ps_av=ps_av, recd=recd, b=b, qh=qh,
                                     hsl=hsl):
                            ps_bc = psS.tile([128, 1024], F32, tag="pss")
                            for half in range(2):
                                nc.tensor.matmul(
                                    ps_bc[0:64, 512 * half:512 * half + 512],
                                    ones_row_r[:, 0:64],
                                    recd[:, 512 * half:512 * half + 512],
                                    start=True, stop=True)
                            avn = avp.tile([64, 1024], BF16, tag="avn")
                            nc.vector.tensor_mul(avn[:], ps_av[0:64, :],
                                                 ps_bc[0:64, :])
                            g0 = 4 * b + 2 * qh
                            nc.sync.dma_start(a2ai[g0, hsl, :], avn[:, 0:TPC])
                            nc.sync.dma_start(a2ai[g0 + 1, hsl, :],
                                              avn[:, TPC:2 * TPC])
                        pending.append(epilogue)

            if pending:
                pending.pop()()

        nc.gpsimd.collective_compute(
            "AllToAll", ALU.bypass, replica_groups=RG,
            ins=[a2ai[:].opt()], outs=[a2ao[:].opt()])
        qkvp_cm.__exit__(None, None, None)   # free Qh/Kh/Vt/invB

        # ============ Phase 4: output projection + residual ============
        with tc.tile_pool(name="avtp", bufs=1) as avtp, \
             tc.tile_pool(name="psA2", bufs=6, space="PSUM") as psA2:
            AVt = avtp.tile([128, NC_N, TPC], BF16)
            nc.scalar.dma_start(AVt[:], a2ao[:].rearrange("s p t -> p s t"))
            for m in range(PO):
                ps = psA2.tile([128, TPC], F32, tag="ps")
                for po in range(PO):
                    nc.tensor.matmul(
                        ps[:], wo_sb[:, po, 128 * m:128 * m + 128],
                        AVt[:, po, :], start=(po == 0), stop=(po == PO - 1))
                nc.vector.scalar_tensor_tensor(
                    out=X2[:, m, :].bitcast(F32), in0=ps[:],
                    scalar=bo[:, m:m + 1], in1=xl[:, m, :],
                    op0=ALU.add, op1=ALU.add)
        xlp_cm.__exit__(None, None, None)    # free xl/wo_sb

        # ============ Phase 5+6: LN2 + MLP ============
        with tc.tile_pool(name="w1p", bufs=4) as w1p, \
             tc.tile_pool(name="w2p", bufs=2) as w2p, \
             tc.tile_pool(name="tmp2", bufs=2) as tmp2p, \
             tc.tile_pool(name="sgp", bufs=2) as sgp, \
             tc.tile_pool(name="outp", bufs=2) as outp, \
             tc.tile_pool(name="psB", bufs=6, space="PSUM") as psB:
            w1_sb = []
            w2_sb = []
            for g in range(4):   # prefetch during the A2A / Wo phase
                w1t = w1p.tile([128, PO, 1024], BF16, tag="w1")
                nc.sync.dma_start(
                    w1t[:], wf1_in.ap()[:, 1024 * g:1024 * (g + 1)]
                    .rearrange("(po p) n -> p po n", p=128))
                w1_sb.append(w1t)
            for g in range(4):
                w2t = w2p.tile([128, M1, 256], BF16, tag="w2")
                nc.sync.dma_start(
                    w2t[:], wf2_in.ap()[:, 256 * g:256 * (g + 1)]
                    .rearrange("(ko p) n -> p ko n", p=128))
                w2_sb.append(w2t)

            ps_s2 = psB.tile([128, TPC], F32, tag="ps")
            for po in range(PO):
                nc.tensor.matmul(ps_s2[0:1, :], ones_col_r[:],
                                 X2[:, po, :],
                                 start=(po == 0), stop=(po == PO - 1))
            ps_q2 = psB.tile([128, TPC], F32, tag="ps")
            for po in range(PO):
                sq = tmp2p.tile([128, TPC], F32R, tag="sq2")
                nc.vector.tensor_mul(sq[:].bitcast(F32),
                                     X2[:, po, :].bitcast(F32),
                                     X2[:, po, :].bitcast(F32))
                nc.tensor.matmul(ps_q2[0:1, :], ones_col_r[:], sq[:],
                                 start=(po == 0), stop=(po == PO - 1))
            nmu2 = rows.tile([1, TPC], F32R, tag="nmu2")
            nc.scalar.activation(nmu2[:].bitcast(F32), ps_s2[0:1, :],
                                 AF.Copy, scale=-1.0 / D)
            ex2b = rows.tile([1, TPC], F32, tag="ex2b")
            nc.scalar.activation(ex2b[:], ps_q2[0:1, :], AF.Copy,
                                 scale=1.0 / D)
            mu2b = rows.tile([1, TPC], F32, tag="mu2b")
            nc.vector.tensor_mul(mu2b[:], nmu2[:].bitcast(F32),
                                 nmu2[:].bitcast(F32))
            varb = rows.tile([1, TPC], F32, tag="varb")
            nc.vector.scalar_tensor_tensor(
                out=varb[:], in0=ex2b[:], scalar=EPS, in1=mu2b[:],
                op0=ALU.add, op1=ALU.subtract)
            recb = rows.tile([1, TPC], F32, tag="recb")
            nc.vector.reciprocal(recb[:], varb[:])
            inv2 = rows.tile([1, TPC], F32R, tag="inv2")
            nc.scalar.activation(inv2[:].bitcast(F32), recb[:], AF.Sqrt)
            ps_mu = psB.tile([128, TPC], F32, tag="ps")
            nc.tensor.matmul(ps_mu[:], ones_row_r[:], nmu2[:],
                             start=True, stop=True)
            ps_iv = psB.tile([128, TPC], F32, tag="ps")
            nc.tensor.matmul(ps_iv[:], ones_row_r[:], inv2[:],
                             start=True, stop=True)
            for po in range(PO):
                t0 = tmp2p.tile([128, TPC], F32, tag="t0")
                nc.vector.tensor_add(t0[:], X2[:, po, :].bitcast(F32),
                                     ps_mu[:])
                nc.vector.tensor_mul(xh2[:, po, :], t0[:], ps_iv[:])

            for m in range(M1):
                ps = psB.tile([128, TPC], F32, tag="ps")
                w1t = w1_sb[m // 8]
                csl = slice(128 * (m % 8), 128 * (m % 8) + 128)
                for po in range(PO):
                    nc.tensor.matmul(ps[:], w1t[:, po, csl],
                                     xh2[:, po, :],
                                     start=(po == 0),
                                     stop=(po == PO - 1))
                sg = sgp.tile([128, TPC], BF16, tag="sg")
                nc.scalar.activation(sg[:], ps[:], AF.Sigmoid,
                                     scale=SIGC,
                                     bias=bf1s[:, m:m + 1])
                nc.vector.scalar_tensor_tensor(
                    out=Amat[:, m, :], in0=ps[:],
                    scalar=bf1[:, m:m + 1], in1=sg[:],
                    op0=ALU.add, op1=ALU.mult)
            out_view = out_t.ap().rearrange("(po p) t -> p po t", p=128)
            for m in range(PO):
                ps = psB.tile([128, TPC], F32, tag="ps")
                w2t = w2_sb[m // 2]
                csl = slice(128 * (m % 2), 128 * (m % 2) + 128)
                for ko in range(M1):
                    nc.tensor.matmul(ps[:], w2t[:, ko, csl],
                                     Amat[:, ko, :],
                                     start=(ko == 0),
                                     stop=(ko == M1 - 1))
                om = outp.tile([128, TPC], F32, tag="om")
                nc.vector.scalar_tensor_tensor(
                    out=om[:], in0=ps[:], scalar=bf2[:, m:m + 1],
                    in1=X2[:, m, :].bitcast(F32),
                    op0=ALU.add, op1=ALU.add)
                nc.sync.dma_start(out_view[:, m, :], om[:])

    nc.compile()
    return nc


def _get_nc():
    key = ("nc", GELU_NATIVE)
    if key not in _CACHE:
        _CACHE[key] = _build()
    return _CACHE[key]


def _make_in_maps(inputs):
    x = np.asarray(inputs["x"], np.float32).reshape(BT, D)
    ln1w = np.asarray(inputs["ln1_w"], np.float32)
    ln1b = np.asarray(inputs["ln1_b"], np.float32)
    ln2w = np.asarray(inputs["ln2_w"], np.float32)
    ln2b = np.asarray(inputs["ln2_b"], np.float32)
    W_qkv0 = np.asarray(inputs["W_qkv"], np.float32)
    W_qkv = W_qkv0 * ln1w[:, None]
    b_qkv = np.asarray(inputs["b_qkv"], np.float32) + ln1b @ W_qkv0
    W_o = np.asarray(inputs["W_o"], np.float32)
    b_o = np.asarray(inputs["b_o"], np.float32)
    W_ff10 = np.asarray(inputs["W_ff1"], np.float32)
    W_ff1 = W_ff10 * ln2w[:, None]
    b_ff1 = np.asarray(inputs["b_ff1"], np.float32) + ln2b @ W_ff10
    W_ff2 = np.asarray(inputs["W_ff2"], np.float32)
    b_ff2 = np.asarray(inputs["b_ff2"], np.float32)

    def pcol(v):  # [D'] -> [128, D'/128] per-partition column layout
        return np.ascontiguousarray(v.reshape(-1, 128).T)

    xT = np.ascontiguousarray(x.T)                      # [D, BT] f32
    common = {
        "xb": xT.astype(NPBF16),
        "wo": W_o.astype(NPBF16),
        "bo": pcol(b_o),
        "wf1": W_ff1.astype(NPBF16),
        "bf1": pcol(b_ff1),
        "bf1s": pcol(SIGC * b_ff1).astype(np.float32),
        "wf2": W_ff2.astype(NPBF16),
        "bf2": pcol(b_ff2),
    }
    in_maps = []
    for r in range(NC_N):
        hc = 128 * r          # first column of this core's Q/K/V head block
        m = dict(common)
        m["xs"] = np.ascontiguousarray(
            xT[:, TPC * r:TPC * (r + 1)]).astype(NPBF16)
        m["xf"] = np.ascontiguousarray(xT[:, TPC * r:TPC * (r + 1)])
        wq = W_qkv[:, hc:hc + 128]
        wk = W_qkv[:, D + hc:D + hc + 128]
        wv = W_qkv[:, 2 * D + hc:2 * D + hc + 128]
        m["wqkv"] = np.ascontiguousarray(
            np.concatenate([wq, wk, wv], axis=1)).astype(NPBF16)
        m["sqkv"] = np.ascontiguousarray(np.concatenate(
            [wq.sum(0), wk.sum(0)]).reshape(1, 256)).astype(np.float32)
        m["svb"] = np.ascontiguousarray(np.stack(
            [wv.sum(0), b_qkv[2 * D + hc:2 * D + hc + 128]])).astype(NPBF16)
        m["bqkv"] = np.ascontiguousarray(np.stack(
            [b_qkv[hc:hc + 128], b_qkv[D + hc:D + hc + 128]], axis=1))
        in_maps.append(m)
    return in_maps


def _run(inputs, trace=False, trace_cores=None):
    nc = _get_nc()
    in_maps = _make_in_maps(inputs)
    res = bass_utils.run_bass_kernel_spmd(
        nc, in_maps, core_ids=list(range(NC_N)), trace=trace,
        trace_cores=trace_cores)
    outs = [res.results[r]["outt"] for r in range(NC_N)]
    full = np.concatenate([o.T for o in outs], axis=0)
    return full.reshape(B, T, D).astype(np.float32), res


def kernel(**inputs):
    out, _ = _run(inputs, trace=False)
    return out
